# revision 31
# baseline (speedup 1.0000x reference)
"""AttnDecoderRNN single-step decoder on 8 Trainium2 NeuronCores.

Sharding:
  - Front (embedding gather, Bahdanau attention, combine+relu, GRU cell):
    data-parallel over batch (32 rows/core).
  - h_new all-gathered (bf16) across the 8 cores.
  - Final fc + log_softmax: tensor-parallel over the vocab dimension
    (6283 columns/core); log-softmax denominators all-gathered and the
    normalization applied locally.

Weights are pre-packed on the host at staging time (transposed to
[in, out] layout and cast to bf16) — a one-time model-load transform.
Activations (hidden, encoder_outputs, tokens) are staged untouched.
"""
import sys

if "/opt/trn_rl_repo" not in sys.path:
    sys.path.insert(0, "/opt/trn_rl_repo")

import numpy as np
import ml_dtypes

import concourse.bass as bass
import concourse.tile as tile
from concourse import bacc, mybir
from concourse import bass_utils
from concourse.masks import make_identity

BF16 = mybir.dt.bfloat16
FP8 = mybir.dt.float8e4
F32 = mybir.dt.float32
I32 = mybir.dt.int32
AF = mybir.ActivationFunctionType

H, V, B, L = 512, 50257, 256, 50
NC = 8
BS = B // NC            # 32 batch rows per core
VS = (V + NC - 1) // NC  # 6283 vocab columns per core
VSP = 6304               # VS padded to a multiple of 32 (SBUF row alignment)
VPAD = VS * NC           # 50264
KT = H // 128            # 4 contraction tiles of 128
RG = [list(range(NC))]

# vocab column tiles (PSUM bank limit: 512 f32 per matmul)
N_TILES = []
_off = 0
while _off < VSP:
    n = min(512, VSP - _off)
    N_TILES.append((_off, n))
    _off += n

_CACHE = {}


def _build(no_bias):
    nc = bacc.Bacc("TRN2", target_bir_lowering=False, debug=False, num_devices=NC)

    # ---- I/O ----
    tok = nc.dram_tensor("tok", [BS, 1], I32, kind="ExternalInput")
    h0 = nc.dram_tensor("h0", [BS, H], F32, kind="ExternalInput")
    enc = nc.dram_tensor("enc", [L, BS, H], F32, kind="ExternalInput")
    embt = nc.dram_tensor("embt", [V, H], BF16, kind="ExternalInput")
    attn_wT = nc.dram_tensor("attn_wT", [2 * H, L], BF16, kind="ExternalInput")
    attn_b = nc.dram_tensor("attn_b", [1, L], BF16, kind="ExternalInput")
    comb_wT = nc.dram_tensor("comb_wT", [2 * H, H], BF16, kind="ExternalInput")
    comb_b = nc.dram_tensor("comb_b", [1, H], BF16, kind="ExternalInput")
    w_ihT = nc.dram_tensor("w_ihT", [H, 3 * H], BF16, kind="ExternalInput")
    w_hhT = nc.dram_tensor("w_hhT", [H, 3 * H], BF16, kind="ExternalInput")
    b_ih = nc.dram_tensor("b_ih", [1, 3 * H], BF16, kind="ExternalInput")
    b_hh = nc.dram_tensor("b_hh", [1, 3 * H], BF16, kind="ExternalInput")
    fc_w8 = nc.dram_tensor("fc_w8", [128, 2, 2, VSP], FP8, kind="ExternalInput")
    fc_b = nc.dram_tensor("fc_b", [1, VSP], BF16, kind="ExternalInput")
    s_corr = nc.dram_tensor("s_corr", [128, 1], F32, kind="ExternalInput")

    out_logp = nc.dram_tensor("out_logp", [B, VS], F32, kind="ExternalOutput")
    out_h = nc.dram_tensor("out_h", [BS, H], F32, kind="ExternalOutput")
    out_attn = nc.dram_tensor("out_attn", [BS, L], F32, kind="ExternalOutput")

    cc_h_in = nc.dram_tensor("cc_h_in", [BS, H], BF16)
    cc_h_out = nc.dram_tensor("cc_h_out", [B, H], BF16, addr_space="Shared")
    cc_s_in = [nc.dram_tensor(f"cc_s_in{bt}", [1, 128], F32) for bt in range(2)]
    cc_s_out = [nc.dram_tensor(f"cc_s_out{bt}", [NC, 128], F32, addr_space="Shared")
                for bt in range(2)]
    cc_d_in = nc.dram_tensor("cc_d_in", [1, 8], F32)
    cc_d_out = nc.dram_tensor("cc_d_out", [NC, 8], F32, addr_space="Shared")

    with tile.TileContext(nc) as tc:
        with (
            tc.tile_pool(name="singles", bufs=1) as sg,
            tc.tile_pool(name="work", bufs=2) as wk,
            tc.tile_pool(name="encp", bufs=3) as encp,
            tc.tile_pool(name="encbp", bufs=4) as encbp,
            tc.tile_pool(name="outp", bufs=3) as outp,
            tc.tile_pool(name="trp", bufs=2, space="PSUM") as trp,
            tc.tile_pool(name="gp", bufs=3, space="PSUM") as gp,
            tc.tile_pool(name="zp", bufs=3, space="PSUM") as zp,
        ):
            # ---- dummy collective: absorbs the expensive first-collective
            # setup concurrently with the front instead of on the h path ----
            dmy = sg.tile([1, 8], F32, tag="dmy")
            nc.vector.memset(dmy[:], 0.0)
            nc.sync.dma_start(out=cc_d_in.ap(), in_=dmy[:])
            nc.gpsimd.collective_compute(
                "AllGather", mybir.AluOpType.bypass, replica_groups=RG,
                ins=[cc_d_in.ap()], outs=[cc_d_out.ap()],
            )

            # ---- constants ----
            id_bf = sg.tile([128, 128], BF16, tag="id_bf")
            make_identity(nc, id_bf[:])
            id_f = sg.tile([128, 128], F32, tag="id_f")
            make_identity(nc, id_f[:])
            ones_bf = sg.tile([1, 128], BF16, tag="ones")
            nc.vector.memset(ones_bf[:], 1.0)

            # ---- encoder outputs: 8 wide DMAs issued first (highest prio),
            # through a 4-deep rolling pool (16 batch rows buffered ahead) ----
            enc_sb = []
            enc_bf = []
            for c in range(8):
                et = encp.tile([L, 4 * H], F32, tag="enc", name=f"enc{c}")
                for half in range(2):
                    nc.sync.dma_start(
                        out=et[:, half * 2 * H:(half + 1) * 2 * H],
                        in_=enc.ap()[:, c * 4 + half * 2:c * 4 + (half + 1) * 2, :]
                        .rearrange("l b h -> l (b h)"))
                eb = encbp.tile([L, 4 * H], BF16, tag="encbf", name=f"encbf{c}")
                if c % 2 == 0:
                    nc.vector.tensor_copy(out=eb[:], in_=et[:])
                else:
                    nc.scalar.copy(out=eb[:], in_=et[:])
                enc_sb.append(et)
                enc_bf.append(eb)

            # ---- embedding gather ----
            tok_sb = sg.tile([BS, 1], I32, tag="tok")
            nc.sync.dma_start(out=tok_sb[:], in_=tok.ap())
            emb_own = sg.tile([BS, H], BF16, tag="embrow")
            nc.gpsimd.indirect_dma_start(
                out=emb_own[:], out_offset=None, in_=embt.ap(),
                in_offset=bass.IndirectOffsetOnAxis(ap=tok_sb[:, 0:1], axis=0),
            )
            h0_sb = sg.tile([BS, H], F32, tag="h0")
            nc.sync.dma_start(out=h0_sb[:], in_=h0.ap())

            # ---- feature-major transposes of embedded and h0 ----
            embT = []
            h0T = []
            for k in range(KT):
                pe = trp.tile([128, BS], BF16, tag="tr")
                nc.tensor.transpose(out=pe[:], in_=emb_own[:, 128 * k:128 * (k + 1)],
                                    identity=id_bf[:BS, :BS])
                t = sg.tile([128, BS], BF16, tag=f"embT{k}")
                nc.vector.tensor_copy(out=t[:], in_=pe[:])
                embT.append(t)

                pf = trp.tile([128, BS], F32, tag="tr")
                nc.tensor.transpose(out=pf[:], in_=h0_sb[:, 128 * k:128 * (k + 1)],
                                    identity=id_f[:BS, :BS])
                t2 = sg.tile([128, BS], BF16, tag=f"h0T{k}")
                nc.vector.tensor_copy(out=t2[:], in_=pf[:])
                h0T.append(t2)

            # ---- attention scores + softmax ----
            aw_sb = sg.tile([128, 2 * KT, L], BF16, tag="aw")
            nc.sync.dma_start(out=aw_sb[:],
                              in_=attn_wT.ap().rearrange("(k p) l -> p k l", p=128))
            ab_sb = sg.tile([1, L], BF16, tag="ab")
            nc.sync.dma_start(out=ab_sb[:], in_=attn_b.ap())

            psc = gp.tile([BS, 512], F32, tag="gpsum")
            for k in range(KT):
                nc.tensor.matmul(out=psc[:, :L], lhsT=embT[k][:], rhs=aw_sb[:, k, :],
                                 start=(k == 0), stop=False)
            for k in range(KT):
                nc.tensor.matmul(out=psc[:, :L], lhsT=h0T[k][:], rhs=aw_sb[:, KT + k, :],
                                 start=False, stop=(no_bias and k == KT - 1))
            if not no_bias:
                nc.tensor.matmul(out=psc[:, :L], lhsT=ones_bf[0:1, :BS], rhs=ab_sb[:],
                                 start=False, stop=True)

            # scores are tiny (|s| < ~1), exp is safe without max subtraction
            e_sb = sg.tile([BS, L], F32, tag="esb")
            ssum = sg.tile([BS, 1], F32, tag="ssum")
            nc.scalar.activation(out=e_sb[:], in_=psc[:, :L], func=AF.Exp,
                                 accum_out=ssum[:])
            rinv = sg.tile([BS, 1], F32, tag="rinv")
            nc.vector.reciprocal(rinv[:], ssum[:])
            # attn_weights output is produced off the critical path; the
            # einsum uses unnormalized exp scores and rescales its output.
            attnw = sg.tile([BS, L], F32, tag="attnw")
            nc.vector.tensor_scalar_mul(attnw[:], e_sb[:], rinv[:, 0:1])
            nc.sync.dma_start(out=out_attn.ap(), in_=attnw[:])

            # ---- attn_applied via masked accumulating matmuls ----
            pwt = trp.tile([L, BS], F32, tag="tr")
            nc.tensor.transpose(out=pwt[:], in_=e_sb[:], identity=id_f[:BS, :BS])
            wt_bf = sg.tile([L, BS], BF16, tag="wtbf")
            nc.vector.tensor_copy(out=wt_bf[:], in_=pwt[:])
            wmask = sg.tile([L, BS, BS], BF16, tag="wmask")
            nc.vector.tensor_copy(
                out=wmask[:],
                in_=wt_bf[:].rearrange("l (o j) -> l o j", o=1).to_broadcast([L, BS, BS]),
            )
            nc.gpsimd.affine_select(
                out=wmask[:], in_=wmask[:], compare_op=mybir.AluOpType.is_equal,
                fill=0.0, base=0, pattern=[[-1, BS], [1, BS]], channel_multiplier=0,
            )
            patt = [gp.tile([BS, 512], F32, tag="gpsum", name=f"patt{i}")
                    for i in range(2)]
            for b2 in range(BS // 2):
                for i in range(2):
                    b = 2 * b2 + i
                    eb = enc_bf[b // 4]
                    bi = b % 4
                    nc.tensor.matmul(out=patt[i][:], lhsT=wmask[:, b, :],
                                     rhs=eb[:, bi * H:(bi + 1) * H],
                                     start=(b2 == 0), stop=(b2 == BS // 2 - 1))
            # merge the two chains; chain0 goes via ACT to SBUF (one PSUM
            # operand max per DVE op), then normalize by the softmax sum
            aa0 = sg.tile([BS, H], F32, tag="aa0")
            nc.scalar.copy(out=aa0[:], in_=patt[0][:])
            aa_s = sg.tile([BS, H], F32, tag="aas")
            nc.vector.tensor_add(out=aa_s[:], in0=aa0[:], in1=patt[1][:])
            aa_bf = sg.tile([BS, H], BF16, tag="aabf")
            nc.vector.tensor_scalar_mul(aa_bf[:], aa_s[:], rinv[:, 0:1])

            aaT = []
            for k in range(KT):
                pe = trp.tile([128, BS], BF16, tag="tr")
                nc.tensor.transpose(out=pe[:], in_=aa_bf[:, 128 * k:128 * (k + 1)],
                                    identity=id_bf[:BS, :BS])
                t = sg.tile([128, BS], BF16, tag=f"aaT{k}")
                nc.vector.tensor_copy(out=t[:], in_=pe[:])
                aaT.append(t)

            # ---- combine + relu ----
            cw_sb = sg.tile([128, 2 * KT, H], BF16, tag="cw")
            nc.sync.dma_start(out=cw_sb[:],
                              in_=comb_wT.ap().rearrange("(k p) n -> p k n", p=128))
            cb_sb = sg.tile([1, H], BF16, tag="cb")
            nc.sync.dma_start(out=cb_sb[:], in_=comb_b.ap())
            px = gp.tile([BS, 512], F32, tag="gpsum")
            for k in range(KT):
                nc.tensor.matmul(out=px[:, :H], lhsT=embT[k][:], rhs=cw_sb[:, k, :],
                                 start=(k == 0), stop=False)
            for k in range(KT):
                nc.tensor.matmul(out=px[:, :H], lhsT=aaT[k][:], rhs=cw_sb[:, KT + k, :],
                                 start=False, stop=(no_bias and k == KT - 1))
            if not no_bias:
                nc.tensor.matmul(out=px[:, :H], lhsT=ones_bf[0:1, :BS], rhs=cb_sb[:],
                                 start=False, stop=True)
            x_bf = sg.tile([BS, H], BF16, tag="xbf")
            nc.scalar.activation(out=x_bf[:], in_=px[:, :H], func=AF.Relu)

            xT = []
            for k in range(KT):
                pe = trp.tile([128, BS], BF16, tag="tr")
                nc.tensor.transpose(out=pe[:], in_=x_bf[:, 128 * k:128 * (k + 1)],
                                    identity=id_bf[:BS, :BS])
                t = sg.tile([128, BS], BF16, tag=f"xT{k}")
                nc.vector.tensor_copy(out=t[:], in_=pe[:])
                xT.append(t)

            # ---- GRU cell ----
            wih_sb = sg.tile([128, KT, 3 * H], BF16, tag="wih")
            nc.sync.dma_start(out=wih_sb[:],
                              in_=w_ihT.ap().rearrange("(k p) n -> p k n", p=128))
            whh_sb = sg.tile([128, KT, 3 * H], BF16, tag="whh")
            nc.sync.dma_start(out=whh_sb[:],
                              in_=w_hhT.ap().rearrange("(k p) n -> p k n", p=128))
            bih_sb = sg.tile([1, 3 * H], BF16, tag="bih")
            nc.sync.dma_start(out=bih_sb[:], in_=b_ih.ap())
            bhh_sb = sg.tile([1, 3 * H], BF16, tag="bhh")
            nc.sync.dma_start(out=bhh_sb[:], in_=b_hh.ap())

            # r and z gates: gi + gh is just a longer matmul accumulation into
            # one PSUM tile; sigmoid reads the PSUM directly.
            r_sb = sg.tile([BS, H], F32, tag="r")
            z_gate = sg.tile([BS, H], F32, tag="zg")
            n_sb = sg.tile([BS, H], F32, tag="n")
            hnew = sg.tile([BS, H], F32, tag="hnew")
            for j, gate_out in ((0, r_sb), (1, z_gate)):
                pg = gp.tile([BS, 512], F32, tag="gpsum")
                for k in range(KT):
                    nc.tensor.matmul(out=pg[:, :H], lhsT=xT[k][:],
                                     rhs=wih_sb[:, k, H * j:H * (j + 1)],
                                     start=(k == 0), stop=False)
                for k in range(KT):
                    nc.tensor.matmul(out=pg[:, :H], lhsT=h0T[k][:],
                                     rhs=whh_sb[:, k, H * j:H * (j + 1)],
                                     start=False, stop=(no_bias and k == KT - 1))
                if not no_bias:
                    nc.tensor.matmul(out=pg[:, :H], lhsT=ones_bf[0:1, :BS],
                                     rhs=bih_sb[:, H * j:H * (j + 1)],
                                     start=False, stop=False)
                    nc.tensor.matmul(out=pg[:, :H], lhsT=ones_bf[0:1, :BS],
                                     rhs=bhh_sb[:, H * j:H * (j + 1)],
                                     start=False, stop=True)
                nc.scalar.activation(out=gate_out[:], in_=pg[:, :H], func=AF.Sigmoid)

            # n gate: i_n and h_n must stay separate (r multiplies h_n only)
            pgi = zp.tile([BS, 512], F32, tag="zpsum", name="pgi_n")
            for k in range(KT):
                nc.tensor.matmul(out=pgi[:, :H], lhsT=xT[k][:],
                                 rhs=wih_sb[:, k, 2 * H:3 * H],
                                 start=(k == 0), stop=(no_bias and k == KT - 1))
            if not no_bias:
                nc.tensor.matmul(out=pgi[:, :H], lhsT=ones_bf[0:1, :BS],
                                 rhs=bih_sb[:, 2 * H:3 * H], start=False, stop=True)
            pgh = gp.tile([BS, 512], F32, tag="gpsum")
            for k in range(KT):
                nc.tensor.matmul(out=pgh[:, :H], lhsT=h0T[k][:],
                                 rhs=whh_sb[:, k, 2 * H:3 * H],
                                 start=(k == 0), stop=(no_bias and k == KT - 1))
            if not no_bias:
                nc.tensor.matmul(out=pgh[:, :H], lhsT=ones_bf[0:1, :BS],
                                 rhs=bhh_sb[:, 2 * H:3 * H], start=False, stop=True)
            hnr = sg.tile([BS, H], F32, tag="hnr")
            nc.vector.tensor_mul(out=hnr[:], in0=r_sb[:], in1=pgh[:, :H])
            pre = sg.tile([BS, H], F32, tag="pre2")
            nc.vector.tensor_add(out=pre[:], in0=hnr[:], in1=pgi[:, :H])
            nc.scalar.activation(out=n_sb[:], in_=pre[:], func=AF.Tanh)

            d_sb = sg.tile([BS, H], F32, tag="d")
            nc.vector.tensor_tensor(out=d_sb[:], in0=h0_sb[:], in1=n_sb[:],
                                    op=mybir.AluOpType.subtract)
            e2_sb = sg.tile([BS, H], F32, tag="e2")
            nc.vector.tensor_mul(out=e2_sb[:], in0=z_gate[:], in1=d_sb[:])
            # final add writes bf16 directly so the AllGather can fire without
            # an extra cast on the critical path; the f32 h_new output is
            # reconstructed from it off-path
            h_bf = sg.tile([BS, H], BF16, tag="hbf")
            nc.vector.tensor_add(out=h_bf[:], in0=n_sb[:], in1=e2_sb[:])
            nc.sync.dma_start(out=cc_h_in.ap(), in_=h_bf[:])
            nc.scalar.activation(out=hnew[:], in_=h_bf[:], func=AF.Copy)
            nc.sync.dma_start(out=out_h.ap(), in_=hnew[:])
            nc.gpsimd.collective_compute(
                "AllGather", mybir.AluOpType.bypass, replica_groups=RG,
                ins=[cc_h_in.ap()], outs=[cc_h_out.ap()],
            )

            hTp = [[sg.tile([128, 2, 128], FP8, tag=f"hTp{p}{bt}", name=f"hTp{p}{bt}")
                    for bt in range(2)] for p in range(2)]
            for bt in range(2):
                hf = wk.tile([128, H], BF16, tag="hfull")
                nc.sync.dma_start(out=hf[:], in_=cc_h_out.ap()[128 * bt:128 * (bt + 1), :])
                for k in range(KT):
                    pe = trp.tile([128, 128], BF16, tag="tr")
                    nc.tensor.transpose(out=pe[:], in_=hf[:, 128 * k:128 * (k + 1)],
                                        identity=id_bf[:])
                    nc.vector.tensor_scalar_mul(hTp[k // 2][bt][:, k % 2, :], pe[:],
                                                1.0 / 16.0)

            # ---- fc matmul + exp/σ stats ----
            # fc_wT is made fully SBUF-resident via 4 big DMAs that carry no
            # dependency on the front, so they stream during front + AllGather.
            if not no_bias:
                fcb_sb = sg.tile([1, VSP], BF16, tag="fcb")
                nc.sync.dma_start(out=fcb_sb[:], in_=fc_b.ap())
            scorr_sb = sg.tile([128, 1], F32, tag="scorr")
            nc.sync.dma_start(out=scorr_sb[:], in_=s_corr.ap())
            wz8 = sg.tile([128, 2, 2, VSP], FP8, tag="wz8")
            for pair in range(2):
                for j in range(2):
                    nc.sync.dma_start(out=wz8[:, pair, j, :],
                                      in_=fc_w8.ap()[:, pair, j, :])
            z_sb = [sg.tile([128, VSP], BF16, tag=f"z{bt}", name=f"z{bt}") for bt in range(2)]
            stats = [sg.tile([128, len(N_TILES)], F32, tag=f"st{bt}", name=f"stats{bt}") for bt in range(2)]

            # bt-outer: batch-tile 0 finishes its matmuls, fires its stats
            # AllGather, and normalizes+stores while batch-tile 1's matmuls
            # are still running on the PE.
            for bt in range(2):
                for ntp in range(0, len(N_TILES), 2):
                    grp = [(nt,) + N_TILES[nt]
                           for nt in range(ntp, min(ntp + 2, len(N_TILES)))]
                    pzs = {nt: zp.tile([128, 512], F32, tag="zpsum",
                                       name=f"pz{bt}_{nt}")
                           for nt, _, _ in grp}
                    for pair in range(2):
                        for nt, ncur, n in grp:
                            nc.tensor.matmul(out=pzs[nt][:, :n],
                                             lhsT=hTp[pair][bt][:],
                                             rhs=wz8[:, pair, :, ncur:ncur + n],
                                             start=(pair == 0),
                                             stop=(no_bias and pair == 1),
                                             perf_mode=mybir.MatmulPerfMode.DoubleRow)
                    for nt, ncur, n in grp:
                        if not no_bias:
                            nc.tensor.matmul(out=pzs[nt][:, :n], lhsT=ones_bf[0:1, :],
                                             rhs=fcb_sb[:, ncur:ncur + n], start=False,
                                             stop=True)
                        nc.vector.tensor_copy(out=z_sb[bt][:, ncur:ncur + n],
                                              in_=pzs[nt][:, :n])
                        esc = wk.tile([128, 512], BF16, tag="esc")
                        nc.scalar.activation(out=esc[:, :n], in_=pzs[nt][:, :n],
                                             func=AF.Exp,
                                             accum_out=stats[bt][:, nt:nt + 1])

                # local softmax denominator for this batch tile -> all-gather
                s_own = sg.tile([128, 1], F32, tag=f"sown{bt}", name=f"sown{bt}")
                nc.vector.reduce_sum(s_own[:], stats[bt][:, 0:len(N_TILES)],
                                     axis=mybir.AxisListType.X)
                if no_bias:
                    # zero-weight pad columns contribute exp(0)=1 each; remove
                    nc.vector.tensor_tensor(out=s_own[:], in0=s_own[:],
                                            in1=scorr_sb[:],
                                            op=mybir.AluOpType.subtract)
                pt = trp.tile([1, 128], F32, tag="tr")
                nc.tensor.transpose(out=pt[:], in_=s_own[:], identity=id_f[:])
                srow = sg.tile([1, 128], F32, tag=f"srow{bt}", name=f"srow{bt}")
                nc.vector.tensor_copy(out=srow[:], in_=pt[:])
                nc.sync.dma_start(out=cc_s_in[bt].ap(), in_=srow[:])
                nc.gpsimd.collective_compute(
                    "AllGather", mybir.AluOpType.bypass, replica_groups=RG,
                    ins=[cc_s_in[bt].ap()], outs=[cc_s_out[bt].ap()],
                )
                s_all = sg.tile([128, NC], F32, tag=f"sall{bt}", name=f"sall{bt}")
                nc.sync.dma_start(out=s_all[:],
                                  in_=cc_s_out[bt].ap().rearrange("r b -> b r"))
                s_tot = sg.tile([128, 1], F32, tag=f"stot{bt}", name=f"stot{bt}")
                nc.vector.reduce_sum(s_tot[:], s_all[:], axis=mybir.AxisListType.X)
                ls = sg.tile([128, 1], F32, tag=f"lse{bt}", name=f"lse{bt}")
                nc.scalar.activation(out=ls[:], in_=s_tot[:], func=AF.Ln)

                # normalize + store this batch tile in wide chunks
                ocur = 0
                while ocur < VS:
                    n = min(1024, VS - ocur)
                    o_t = outp.tile([128, 1024], F32, tag="ost")
                    nc.vector.tensor_scalar_sub(o_t[:, :n], z_sb[bt][:, ocur:ocur + n],
                                                ls[:, 0:1])
                    nc.sync.dma_start(
                        out=out_logp.ap()[128 * bt:128 * (bt + 1), ocur:ocur + n],
                        in_=o_t[:, :n])
                    ocur += n

    nc.compile()
    return nc


def _pack_fp8(wT):
    # wT [512, VSP] f32 -> [128, 2, 2, VSP] fp8e4m3, x16 scaling
    # (matmul uses h/16 so the scales cancel exactly in the f32 PSUM)
    arr = (wT * 16.0).reshape(2, 2, 128, wT.shape[1])  # [pair, j, ki, v]
    return np.ascontiguousarray(arr.transpose(2, 0, 1, 3)).astype(
        ml_dtypes.float8_e4m3)


def _pad_cols(a, w, fill=0.0):
    out = np.full((a.shape[0], w), fill, np.float32)
    out[:, :a.shape[1]] = a
    return out


def _stage(inputs):
    """Build the 8 per-core in_maps from the full-size inputs."""
    bf = ml_dtypes.bfloat16
    tok = np.asarray(inputs["input_tokens"]).astype(np.int32).reshape(B, 1)
    hidden = np.ascontiguousarray(np.asarray(inputs["hidden"], np.float32))[0]  # [B,H]
    enc = np.ascontiguousarray(np.asarray(inputs["encoder_outputs"], np.float32))
    emb_bf = np.asarray(inputs["emb"], np.float32).astype(bf)
    attn_wT = np.ascontiguousarray(np.asarray(inputs["attn_w"], np.float32).T).astype(bf)
    attn_b = np.asarray(inputs["attn_b"], np.float32).reshape(1, L).astype(bf)
    comb_wT = np.ascontiguousarray(np.asarray(inputs["comb_w"], np.float32).T).astype(bf)
    comb_b = np.asarray(inputs["comb_b"], np.float32).reshape(1, H).astype(bf)
    w_ihT = np.ascontiguousarray(np.asarray(inputs["w_ih"], np.float32).T).astype(bf)
    w_hhT = np.ascontiguousarray(np.asarray(inputs["w_hh"], np.float32).T).astype(bf)
    b_ih = np.asarray(inputs["b_ih"], np.float32).reshape(1, 3 * H).astype(bf)
    b_hh = np.asarray(inputs["b_hh"], np.float32).reshape(1, 3 * H).astype(bf)

    fc_w = np.asarray(inputs["fc_w"], np.float32)
    fc_b = np.asarray(inputs["fc_b"], np.float32)
    fc_w_pad = np.zeros((VPAD, H), np.float32)
    fc_w_pad[:V] = fc_w
    fc_b_pad = np.full((VPAD,), -1e30, np.float32)
    fc_b_pad[:V] = fc_b

    in_maps = []
    for c in range(NC):
        b0 = c * BS
        v0 = c * VS
        in_maps.append({
            "tok": tok[b0:b0 + BS],
            "h0": np.ascontiguousarray(hidden[b0:b0 + BS]),
            "enc": np.ascontiguousarray(enc[:, b0:b0 + BS, :]),
            "embt": emb_bf,
            "attn_wT": attn_wT,
            "attn_b": attn_b,
            "comb_wT": comb_wT,
            "comb_b": comb_b,
            "w_ihT": w_ihT,
            "w_hhT": w_hhT,
            "b_ih": b_ih,
            "b_hh": b_hh,
            "fc_w8": _pack_fp8(_pad_cols(np.ascontiguousarray(fc_w_pad[v0:v0 + VS].T), VSP)),
            "fc_b": _pad_cols(fc_b_pad[v0:v0 + VS].reshape(1, VS), VSP, fill=-1e30).astype(bf),
        })
    return in_maps


def _run(inputs, trace=False, trace_cores=None):
    no_bias = all(
        not np.any(np.asarray(inputs[k]))
        for k in ("attn_b", "comb_b", "b_ih", "b_hh", "fc_b"))
    key = ("nc", no_bias)
    if key not in _CACHE:
        _CACHE[key] = _build(no_bias)
    nc = _CACHE[key]
    in_maps = _stage(inputs)
    for c in range(NC):
        v0 = c * VS
        n_real = max(0, min(V - v0, VS))
        in_maps[c]["s_corr"] = np.full((128, 1), float(VSP - n_real)
                                       if no_bias else 0.0, np.float32)
        if no_bias:
            # pad columns rely on the s_corr subtraction, not a -inf bias
            in_maps[c]["fc_b"] = np.zeros_like(in_maps[c]["fc_b"])
    res = bass_utils.run_bass_kernel_spmd(
        nc, in_maps, core_ids=list(range(NC)), trace=trace, trace_cores=trace_cores)
    logp = np.concatenate([res.results[c]["out_logp"] for c in range(NC)], axis=1)[:, :V]
    h_new = np.concatenate([res.results[c]["out_h"] for c in range(NC)], axis=0)[None]
    attnw = np.concatenate([res.results[c]["out_attn"] for c in range(NC)], axis=0)
    return (logp, h_new, attnw), res


def kernel(**inputs):
    out, _ = _run(inputs, trace=False)
    return out


# revision 32
# speedup vs baseline: 1.0571x; 1.0571x over previous
"""AttnDecoderRNN single-step decoder on 8 Trainium2 NeuronCores.

Sharding:
  - Front (embedding gather, Bahdanau attention, combine+relu, GRU cell):
    data-parallel over batch (32 rows/core).
  - h_new all-gathered (bf16) across the 8 cores.
  - Final fc + log_softmax: tensor-parallel over the vocab dimension
    (6283 columns/core); log-softmax denominators all-gathered and the
    normalization applied locally.

Weights are pre-packed on the host at staging time (transposed to
[in, out] layout and cast to bf16) — a one-time model-load transform.
Activations (hidden, encoder_outputs, tokens) are staged untouched.
"""
import sys

if "/opt/trn_rl_repo" not in sys.path:
    sys.path.insert(0, "/opt/trn_rl_repo")

import numpy as np
import ml_dtypes

import concourse.bass as bass
import concourse.tile as tile
from concourse import bacc, mybir
from concourse import bass_utils
from concourse.masks import make_identity

BF16 = mybir.dt.bfloat16
FP8 = mybir.dt.float8e4
F32 = mybir.dt.float32
I32 = mybir.dt.int32
AF = mybir.ActivationFunctionType

H, V, B, L = 512, 50257, 256, 50
NC = 8
BS = B // NC            # 32 batch rows per core
VS = (V + NC - 1) // NC  # 6283 vocab columns per core
VSP = 6304               # VS padded to a multiple of 32 (SBUF row alignment)
VPAD = VS * NC           # 50264
KT = H // 128            # 4 contraction tiles of 128
RG = [list(range(NC))]

# vocab column tiles (PSUM bank limit: 512 f32 per matmul)
N_TILES = []
_off = 0
while _off < VSP:
    n = min(512, VSP - _off)
    N_TILES.append((_off, n))
    _off += n

_CACHE = {}


def _build(no_bias):
    nc = bacc.Bacc("TRN2", target_bir_lowering=False, debug=False, num_devices=NC)

    # ---- I/O ----
    tok = nc.dram_tensor("tok", [BS, 1], I32, kind="ExternalInput")
    h0 = nc.dram_tensor("h0", [BS, H], F32, kind="ExternalInput")
    enc = nc.dram_tensor("enc", [L, BS, H], F32, kind="ExternalInput")
    embt = nc.dram_tensor("embt", [V, H], BF16, kind="ExternalInput")
    attn_wT = nc.dram_tensor("attn_wT", [2 * H, L], BF16, kind="ExternalInput")
    attn_b = nc.dram_tensor("attn_b", [1, L], BF16, kind="ExternalInput")
    comb_wT = nc.dram_tensor("comb_wT", [2 * H, H], BF16, kind="ExternalInput")
    comb_b = nc.dram_tensor("comb_b", [1, H], BF16, kind="ExternalInput")
    w_ihT = nc.dram_tensor("w_ihT", [H, 3 * H], BF16, kind="ExternalInput")
    w_hhT = nc.dram_tensor("w_hhT", [H, 3 * H], BF16, kind="ExternalInput")
    b_ih = nc.dram_tensor("b_ih", [1, 3 * H], BF16, kind="ExternalInput")
    b_hh = nc.dram_tensor("b_hh", [1, 3 * H], BF16, kind="ExternalInput")
    fc_w8 = nc.dram_tensor("fc_w8", [128, 2, 2, VSP], FP8, kind="ExternalInput")
    fc_b = nc.dram_tensor("fc_b", [1, VSP], BF16, kind="ExternalInput")
    s_corr = nc.dram_tensor("s_corr", [128, 1], F32, kind="ExternalInput")

    out_logp = nc.dram_tensor("out_logp", [B, VS], F32, kind="ExternalOutput")
    out_h = nc.dram_tensor("out_h", [BS, H], F32, kind="ExternalOutput")
    out_attn = nc.dram_tensor("out_attn", [BS, L], F32, kind="ExternalOutput")

    cc_h_in = nc.dram_tensor("cc_h_in", [BS, H], BF16)
    cc_h_out = nc.dram_tensor("cc_h_out", [B, H], BF16, addr_space="Shared")
    cc_s_in = [nc.dram_tensor(f"cc_s_in{bt}", [1, 128], F32) for bt in range(2)]
    cc_s_out = [nc.dram_tensor(f"cc_s_out{bt}", [NC, 128], F32, addr_space="Shared")
                for bt in range(2)]
    cc_d_in = nc.dram_tensor("cc_d_in", [1, 8], F32)
    cc_d_out = nc.dram_tensor("cc_d_out", [NC, 8], F32, addr_space="Shared")

    with tile.TileContext(nc) as tc:
        with (
            tc.tile_pool(name="singles", bufs=1) as sg,
            tc.tile_pool(name="work", bufs=2) as wk,
            tc.tile_pool(name="encp", bufs=3) as encp,
            tc.tile_pool(name="encbp", bufs=4) as encbp,
            tc.tile_pool(name="outp", bufs=3) as outp,
            tc.tile_pool(name="trp", bufs=2, space="PSUM") as trp,
            tc.tile_pool(name="gp", bufs=3, space="PSUM") as gp,
            tc.tile_pool(name="zp", bufs=3, space="PSUM") as zp,
        ):
            # ---- constants ----
            id_bf = sg.tile([128, 128], BF16, tag="id_bf")
            make_identity(nc, id_bf[:])
            id_f = sg.tile([128, 128], F32, tag="id_f")
            make_identity(nc, id_f[:])
            ones_bf = sg.tile([1, 128], BF16, tag="ones")
            nc.vector.memset(ones_bf[:], 1.0)

            # ---- encoder outputs: 8 wide DMAs issued first (highest prio),
            # through a 4-deep rolling pool (16 batch rows buffered ahead) ----
            enc_sb = []
            enc_bf = []
            for c in range(8):
                et = encp.tile([L, 4 * H], F32, tag="enc", name=f"enc{c}")
                for half in range(2):
                    nc.sync.dma_start(
                        out=et[:, half * 2 * H:(half + 1) * 2 * H],
                        in_=enc.ap()[:, c * 4 + half * 2:c * 4 + (half + 1) * 2, :]
                        .rearrange("l b h -> l (b h)"))
                eb = encbp.tile([L, 4 * H], BF16, tag="encbf", name=f"encbf{c}")
                if c % 2 == 0:
                    nc.vector.tensor_copy(out=eb[:], in_=et[:])
                else:
                    nc.scalar.copy(out=eb[:], in_=et[:])
                enc_sb.append(et)
                enc_bf.append(eb)

            # ---- embedding gather ----
            tok_sb = sg.tile([BS, 1], I32, tag="tok")
            nc.sync.dma_start(out=tok_sb[:], in_=tok.ap())
            emb_own = sg.tile([BS, H], BF16, tag="embrow")
            nc.gpsimd.indirect_dma_start(
                out=emb_own[:], out_offset=None, in_=embt.ap(),
                in_offset=bass.IndirectOffsetOnAxis(ap=tok_sb[:, 0:1], axis=0),
            )
            h0_sb = sg.tile([BS, H], F32, tag="h0")
            nc.sync.dma_start(out=h0_sb[:], in_=h0.ap())

            # ---- feature-major transposes of embedded and h0 ----
            embT = []
            h0T = []
            for k in range(KT):
                pe = trp.tile([128, BS], BF16, tag="tr")
                nc.tensor.transpose(out=pe[:], in_=emb_own[:, 128 * k:128 * (k + 1)],
                                    identity=id_bf[:BS, :BS])
                t = sg.tile([128, BS], BF16, tag=f"embT{k}")
                nc.vector.tensor_copy(out=t[:], in_=pe[:])
                embT.append(t)

                pf = trp.tile([128, BS], F32, tag="tr")
                nc.tensor.transpose(out=pf[:], in_=h0_sb[:, 128 * k:128 * (k + 1)],
                                    identity=id_f[:BS, :BS])
                t2 = sg.tile([128, BS], BF16, tag=f"h0T{k}")
                nc.vector.tensor_copy(out=t2[:], in_=pf[:])
                h0T.append(t2)

            # ---- attention scores + softmax ----
            aw_sb = sg.tile([128, 2 * KT, L], BF16, tag="aw")
            nc.sync.dma_start(out=aw_sb[:],
                              in_=attn_wT.ap().rearrange("(k p) l -> p k l", p=128))
            ab_sb = sg.tile([1, L], BF16, tag="ab")
            nc.sync.dma_start(out=ab_sb[:], in_=attn_b.ap())

            psc = gp.tile([BS, 512], F32, tag="gpsum")
            for k in range(KT):
                nc.tensor.matmul(out=psc[:, :L], lhsT=embT[k][:], rhs=aw_sb[:, k, :],
                                 start=(k == 0), stop=False)
            for k in range(KT):
                nc.tensor.matmul(out=psc[:, :L], lhsT=h0T[k][:], rhs=aw_sb[:, KT + k, :],
                                 start=False, stop=(no_bias and k == KT - 1))
            if not no_bias:
                nc.tensor.matmul(out=psc[:, :L], lhsT=ones_bf[0:1, :BS], rhs=ab_sb[:],
                                 start=False, stop=True)

            # scores are tiny (|s| < ~1), exp is safe without max subtraction
            e_sb = sg.tile([BS, L], F32, tag="esb")
            ssum = sg.tile([BS, 1], F32, tag="ssum")
            nc.scalar.activation(out=e_sb[:], in_=psc[:, :L], func=AF.Exp,
                                 accum_out=ssum[:])
            rinv = sg.tile([BS, 1], F32, tag="rinv")
            nc.vector.reciprocal(rinv[:], ssum[:])
            # attn_weights output is produced off the critical path; the
            # einsum uses unnormalized exp scores and rescales its output.
            attnw = sg.tile([BS, L], F32, tag="attnw")
            nc.vector.tensor_scalar_mul(attnw[:], e_sb[:], rinv[:, 0:1])
            nc.sync.dma_start(out=out_attn.ap(), in_=attnw[:])

            # ---- attn_applied via masked accumulating matmuls ----
            pwt = trp.tile([L, BS], F32, tag="tr")
            nc.tensor.transpose(out=pwt[:], in_=e_sb[:], identity=id_f[:BS, :BS])
            wt_bf = sg.tile([L, BS], BF16, tag="wtbf")
            nc.vector.tensor_copy(out=wt_bf[:], in_=pwt[:])
            wmask = sg.tile([L, BS, BS], BF16, tag="wmask")
            nc.vector.tensor_copy(
                out=wmask[:],
                in_=wt_bf[:].rearrange("l (o j) -> l o j", o=1).to_broadcast([L, BS, BS]),
            )
            nc.gpsimd.affine_select(
                out=wmask[:], in_=wmask[:], compare_op=mybir.AluOpType.is_equal,
                fill=0.0, base=0, pattern=[[-1, BS], [1, BS]], channel_multiplier=0,
            )
            # dummy collective: absorbs the expensive first-collective setup
            # while the einsum/comb/GRU run, instead of on the h path. Placed
            # after the last front gpsimd op — the gpsimd FIFO blocks behind
            # a collective's completion wait.
            dmy = sg.tile([1, 8], F32, tag="dmy")
            nc.vector.memset(dmy[:], 0.0)
            nc.sync.dma_start(out=cc_d_in.ap(), in_=dmy[:])
            nc.gpsimd.collective_compute(
                "AllGather", mybir.AluOpType.bypass, replica_groups=RG,
                ins=[cc_d_in.ap()], outs=[cc_d_out.ap()],
            )
            patt = [gp.tile([BS, 512], F32, tag="gpsum", name=f"patt{i}")
                    for i in range(2)]
            for b2 in range(BS // 2):
                for i in range(2):
                    b = 2 * b2 + i
                    eb = enc_bf[b // 4]
                    bi = b % 4
                    nc.tensor.matmul(out=patt[i][:], lhsT=wmask[:, b, :],
                                     rhs=eb[:, bi * H:(bi + 1) * H],
                                     start=(b2 == 0), stop=(b2 == BS // 2 - 1))
            # merge the two chains; chain0 goes via ACT to SBUF (one PSUM
            # operand max per DVE op), then normalize by the softmax sum
            aa0 = sg.tile([BS, H], F32, tag="aa0")
            nc.scalar.copy(out=aa0[:], in_=patt[0][:])
            aa_s = sg.tile([BS, H], F32, tag="aas")
            nc.vector.tensor_add(out=aa_s[:], in0=aa0[:], in1=patt[1][:])
            aa_bf = sg.tile([BS, H], BF16, tag="aabf")
            nc.vector.tensor_scalar_mul(aa_bf[:], aa_s[:], rinv[:, 0:1])

            aaT = []
            for k in range(KT):
                pe = trp.tile([128, BS], BF16, tag="tr")
                nc.tensor.transpose(out=pe[:], in_=aa_bf[:, 128 * k:128 * (k + 1)],
                                    identity=id_bf[:BS, :BS])
                t = sg.tile([128, BS], BF16, tag=f"aaT{k}")
                nc.vector.tensor_copy(out=t[:], in_=pe[:])
                aaT.append(t)

            # ---- combine + relu ----
            cw_sb = sg.tile([128, 2 * KT, H], BF16, tag="cw")
            nc.sync.dma_start(out=cw_sb[:],
                              in_=comb_wT.ap().rearrange("(k p) n -> p k n", p=128))
            cb_sb = sg.tile([1, H], BF16, tag="cb")
            nc.sync.dma_start(out=cb_sb[:], in_=comb_b.ap())
            px = gp.tile([BS, 512], F32, tag="gpsum")
            for k in range(KT):
                nc.tensor.matmul(out=px[:, :H], lhsT=embT[k][:], rhs=cw_sb[:, k, :],
                                 start=(k == 0), stop=False)
            for k in range(KT):
                nc.tensor.matmul(out=px[:, :H], lhsT=aaT[k][:], rhs=cw_sb[:, KT + k, :],
                                 start=False, stop=(no_bias and k == KT - 1))
            if not no_bias:
                nc.tensor.matmul(out=px[:, :H], lhsT=ones_bf[0:1, :BS], rhs=cb_sb[:],
                                 start=False, stop=True)
            x_bf = sg.tile([BS, H], BF16, tag="xbf")
            nc.scalar.activation(out=x_bf[:], in_=px[:, :H], func=AF.Relu)

            xT = []
            for k in range(KT):
                pe = trp.tile([128, BS], BF16, tag="tr")
                nc.tensor.transpose(out=pe[:], in_=x_bf[:, 128 * k:128 * (k + 1)],
                                    identity=id_bf[:BS, :BS])
                t = sg.tile([128, BS], BF16, tag=f"xT{k}")
                nc.vector.tensor_copy(out=t[:], in_=pe[:])
                xT.append(t)

            # ---- GRU cell ----
            wih_sb = sg.tile([128, KT, 3 * H], BF16, tag="wih")
            nc.sync.dma_start(out=wih_sb[:],
                              in_=w_ihT.ap().rearrange("(k p) n -> p k n", p=128))
            whh_sb = sg.tile([128, KT, 3 * H], BF16, tag="whh")
            nc.sync.dma_start(out=whh_sb[:],
                              in_=w_hhT.ap().rearrange("(k p) n -> p k n", p=128))
            bih_sb = sg.tile([1, 3 * H], BF16, tag="bih")
            nc.sync.dma_start(out=bih_sb[:], in_=b_ih.ap())
            bhh_sb = sg.tile([1, 3 * H], BF16, tag="bhh")
            nc.sync.dma_start(out=bhh_sb[:], in_=b_hh.ap())

            # r and z gates: gi + gh is just a longer matmul accumulation into
            # one PSUM tile; sigmoid reads the PSUM directly.
            r_sb = sg.tile([BS, H], F32, tag="r")
            z_gate = sg.tile([BS, H], F32, tag="zg")
            n_sb = sg.tile([BS, H], F32, tag="n")
            hnew = sg.tile([BS, H], F32, tag="hnew")
            for j, gate_out in ((0, r_sb), (1, z_gate)):
                pg = gp.tile([BS, 512], F32, tag="gpsum")
                for k in range(KT):
                    nc.tensor.matmul(out=pg[:, :H], lhsT=xT[k][:],
                                     rhs=wih_sb[:, k, H * j:H * (j + 1)],
                                     start=(k == 0), stop=False)
                for k in range(KT):
                    nc.tensor.matmul(out=pg[:, :H], lhsT=h0T[k][:],
                                     rhs=whh_sb[:, k, H * j:H * (j + 1)],
                                     start=False, stop=(no_bias and k == KT - 1))
                if not no_bias:
                    nc.tensor.matmul(out=pg[:, :H], lhsT=ones_bf[0:1, :BS],
                                     rhs=bih_sb[:, H * j:H * (j + 1)],
                                     start=False, stop=False)
                    nc.tensor.matmul(out=pg[:, :H], lhsT=ones_bf[0:1, :BS],
                                     rhs=bhh_sb[:, H * j:H * (j + 1)],
                                     start=False, stop=True)
                nc.scalar.activation(out=gate_out[:], in_=pg[:, :H], func=AF.Sigmoid)

            # n gate: i_n and h_n must stay separate (r multiplies h_n only)
            pgi = zp.tile([BS, 512], F32, tag="zpsum", name="pgi_n")
            for k in range(KT):
                nc.tensor.matmul(out=pgi[:, :H], lhsT=xT[k][:],
                                 rhs=wih_sb[:, k, 2 * H:3 * H],
                                 start=(k == 0), stop=(no_bias and k == KT - 1))
            if not no_bias:
                nc.tensor.matmul(out=pgi[:, :H], lhsT=ones_bf[0:1, :BS],
                                 rhs=bih_sb[:, 2 * H:3 * H], start=False, stop=True)
            pgh = gp.tile([BS, 512], F32, tag="gpsum")
            for k in range(KT):
                nc.tensor.matmul(out=pgh[:, :H], lhsT=h0T[k][:],
                                 rhs=whh_sb[:, k, 2 * H:3 * H],
                                 start=(k == 0), stop=(no_bias and k == KT - 1))
            if not no_bias:
                nc.tensor.matmul(out=pgh[:, :H], lhsT=ones_bf[0:1, :BS],
                                 rhs=bhh_sb[:, 2 * H:3 * H], start=False, stop=True)
            hnr = sg.tile([BS, H], F32, tag="hnr")
            nc.vector.tensor_mul(out=hnr[:], in0=r_sb[:], in1=pgh[:, :H])
            pre = sg.tile([BS, H], F32, tag="pre2")
            nc.vector.tensor_add(out=pre[:], in0=hnr[:], in1=pgi[:, :H])
            nc.scalar.activation(out=n_sb[:], in_=pre[:], func=AF.Tanh)

            d_sb = sg.tile([BS, H], F32, tag="d")
            nc.vector.tensor_tensor(out=d_sb[:], in0=h0_sb[:], in1=n_sb[:],
                                    op=mybir.AluOpType.subtract)
            e2_sb = sg.tile([BS, H], F32, tag="e2")
            nc.vector.tensor_mul(out=e2_sb[:], in0=z_gate[:], in1=d_sb[:])
            # final add writes bf16 directly so the AllGather can fire without
            # an extra cast on the critical path; the f32 h_new output is
            # reconstructed from it off-path
            h_bf = sg.tile([BS, H], BF16, tag="hbf")
            nc.vector.tensor_add(out=h_bf[:], in0=n_sb[:], in1=e2_sb[:])
            nc.sync.dma_start(out=cc_h_in.ap(), in_=h_bf[:])
            nc.scalar.activation(out=hnew[:], in_=h_bf[:], func=AF.Copy)
            nc.sync.dma_start(out=out_h.ap(), in_=hnew[:])
            nc.gpsimd.collective_compute(
                "AllGather", mybir.AluOpType.bypass, replica_groups=RG,
                ins=[cc_h_in.ap()], outs=[cc_h_out.ap()],
            )

            hTp = [[sg.tile([128, 2, 128], FP8, tag=f"hTp{p}{bt}", name=f"hTp{p}{bt}")
                    for bt in range(2)] for p in range(2)]
            for bt in range(2):
                hf = wk.tile([128, H], BF16, tag="hfull")
                nc.sync.dma_start(out=hf[:], in_=cc_h_out.ap()[128 * bt:128 * (bt + 1), :])
                for k in range(KT):
                    pe = trp.tile([128, 128], BF16, tag="tr")
                    nc.tensor.transpose(out=pe[:], in_=hf[:, 128 * k:128 * (k + 1)],
                                        identity=id_bf[:])
                    nc.vector.tensor_scalar_mul(hTp[k // 2][bt][:, k % 2, :], pe[:],
                                                1.0 / 16.0)

            # ---- fc matmul + exp/σ stats ----
            # fc_wT is made fully SBUF-resident via 4 big DMAs that carry no
            # dependency on the front, so they stream during front + AllGather.
            if not no_bias:
                fcb_sb = sg.tile([1, VSP], BF16, tag="fcb")
                nc.sync.dma_start(out=fcb_sb[:], in_=fc_b.ap())
            scorr_sb = sg.tile([128, 1], F32, tag="scorr")
            nc.sync.dma_start(out=scorr_sb[:], in_=s_corr.ap())
            wz8 = sg.tile([128, 2, 2, VSP], FP8, tag="wz8")
            for pair in range(2):
                for j in range(2):
                    nc.sync.dma_start(out=wz8[:, pair, j, :],
                                      in_=fc_w8.ap()[:, pair, j, :])
            z_sb = [sg.tile([128, VSP], BF16, tag=f"z{bt}", name=f"z{bt}") for bt in range(2)]
            stats = [sg.tile([128, len(N_TILES)], F32, tag=f"st{bt}", name=f"stats{bt}") for bt in range(2)]

            # bt-outer: batch-tile 0 finishes its matmuls, fires its stats
            # AllGather, and normalizes+stores while batch-tile 1's matmuls
            # are still running on the PE.
            for bt in range(2):
                for ntp in range(0, len(N_TILES), 2):
                    grp = [(nt,) + N_TILES[nt]
                           for nt in range(ntp, min(ntp + 2, len(N_TILES)))]
                    pzs = {nt: zp.tile([128, 512], F32, tag="zpsum",
                                       name=f"pz{bt}_{nt}")
                           for nt, _, _ in grp}
                    for pair in range(2):
                        for nt, ncur, n in grp:
                            nc.tensor.matmul(out=pzs[nt][:, :n],
                                             lhsT=hTp[pair][bt][:],
                                             rhs=wz8[:, pair, :, ncur:ncur + n],
                                             start=(pair == 0),
                                             stop=(no_bias and pair == 1),
                                             perf_mode=mybir.MatmulPerfMode.DoubleRow)
                    for nt, ncur, n in grp:
                        if not no_bias:
                            nc.tensor.matmul(out=pzs[nt][:, :n], lhsT=ones_bf[0:1, :],
                                             rhs=fcb_sb[:, ncur:ncur + n], start=False,
                                             stop=True)
                        nc.vector.tensor_copy(out=z_sb[bt][:, ncur:ncur + n],
                                              in_=pzs[nt][:, :n])
                        esc = wk.tile([128, 512], BF16, tag="esc")
                        nc.scalar.activation(out=esc[:, :n], in_=pzs[nt][:, :n],
                                             func=AF.Exp,
                                             accum_out=stats[bt][:, nt:nt + 1])

                # local softmax denominator for this batch tile -> all-gather
                s_own = sg.tile([128, 1], F32, tag=f"sown{bt}", name=f"sown{bt}")
                nc.vector.reduce_sum(s_own[:], stats[bt][:, 0:len(N_TILES)],
                                     axis=mybir.AxisListType.X)
                if no_bias:
                    # zero-weight pad columns contribute exp(0)=1 each; remove
                    nc.vector.tensor_tensor(out=s_own[:], in0=s_own[:],
                                            in1=scorr_sb[:],
                                            op=mybir.AluOpType.subtract)
                pt = trp.tile([1, 128], F32, tag="tr")
                nc.tensor.transpose(out=pt[:], in_=s_own[:], identity=id_f[:])
                srow = sg.tile([1, 128], F32, tag=f"srow{bt}", name=f"srow{bt}")
                nc.vector.tensor_copy(out=srow[:], in_=pt[:])
                nc.sync.dma_start(out=cc_s_in[bt].ap(), in_=srow[:])
                nc.gpsimd.collective_compute(
                    "AllGather", mybir.AluOpType.bypass, replica_groups=RG,
                    ins=[cc_s_in[bt].ap()], outs=[cc_s_out[bt].ap()],
                )
                s_all = sg.tile([128, NC], F32, tag=f"sall{bt}", name=f"sall{bt}")
                nc.sync.dma_start(out=s_all[:],
                                  in_=cc_s_out[bt].ap().rearrange("r b -> b r"))
                s_tot = sg.tile([128, 1], F32, tag=f"stot{bt}", name=f"stot{bt}")
                nc.vector.reduce_sum(s_tot[:], s_all[:], axis=mybir.AxisListType.X)
                ls = sg.tile([128, 1], F32, tag=f"lse{bt}", name=f"lse{bt}")
                nc.scalar.activation(out=ls[:], in_=s_tot[:], func=AF.Ln)

                # normalize + store this batch tile in wide chunks
                ocur = 0
                while ocur < VS:
                    n = min(1024, VS - ocur)
                    o_t = outp.tile([128, 1024], F32, tag="ost")
                    nc.vector.tensor_scalar_sub(o_t[:, :n], z_sb[bt][:, ocur:ocur + n],
                                                ls[:, 0:1])
                    nc.sync.dma_start(
                        out=out_logp.ap()[128 * bt:128 * (bt + 1), ocur:ocur + n],
                        in_=o_t[:, :n])
                    ocur += n

    nc.compile()
    return nc


def _pack_fp8(wT):
    # wT [512, VSP] f32 -> [128, 2, 2, VSP] fp8e4m3, x16 scaling
    # (matmul uses h/16 so the scales cancel exactly in the f32 PSUM)
    arr = (wT * 16.0).reshape(2, 2, 128, wT.shape[1])  # [pair, j, ki, v]
    return np.ascontiguousarray(arr.transpose(2, 0, 1, 3)).astype(
        ml_dtypes.float8_e4m3)


def _pad_cols(a, w, fill=0.0):
    out = np.full((a.shape[0], w), fill, np.float32)
    out[:, :a.shape[1]] = a
    return out


def _stage(inputs):
    """Build the 8 per-core in_maps from the full-size inputs."""
    bf = ml_dtypes.bfloat16
    tok = np.asarray(inputs["input_tokens"]).astype(np.int32).reshape(B, 1)
    hidden = np.ascontiguousarray(np.asarray(inputs["hidden"], np.float32))[0]  # [B,H]
    enc = np.ascontiguousarray(np.asarray(inputs["encoder_outputs"], np.float32))
    emb_bf = np.asarray(inputs["emb"], np.float32).astype(bf)
    attn_wT = np.ascontiguousarray(np.asarray(inputs["attn_w"], np.float32).T).astype(bf)
    attn_b = np.asarray(inputs["attn_b"], np.float32).reshape(1, L).astype(bf)
    comb_wT = np.ascontiguousarray(np.asarray(inputs["comb_w"], np.float32).T).astype(bf)
    comb_b = np.asarray(inputs["comb_b"], np.float32).reshape(1, H).astype(bf)
    w_ihT = np.ascontiguousarray(np.asarray(inputs["w_ih"], np.float32).T).astype(bf)
    w_hhT = np.ascontiguousarray(np.asarray(inputs["w_hh"], np.float32).T).astype(bf)
    b_ih = np.asarray(inputs["b_ih"], np.float32).reshape(1, 3 * H).astype(bf)
    b_hh = np.asarray(inputs["b_hh"], np.float32).reshape(1, 3 * H).astype(bf)

    fc_w = np.asarray(inputs["fc_w"], np.float32)
    fc_b = np.asarray(inputs["fc_b"], np.float32)
    fc_w_pad = np.zeros((VPAD, H), np.float32)
    fc_w_pad[:V] = fc_w
    fc_b_pad = np.full((VPAD,), -1e30, np.float32)
    fc_b_pad[:V] = fc_b

    in_maps = []
    for c in range(NC):
        b0 = c * BS
        v0 = c * VS
        in_maps.append({
            "tok": tok[b0:b0 + BS],
            "h0": np.ascontiguousarray(hidden[b0:b0 + BS]),
            "enc": np.ascontiguousarray(enc[:, b0:b0 + BS, :]),
            "embt": emb_bf,
            "attn_wT": attn_wT,
            "attn_b": attn_b,
            "comb_wT": comb_wT,
            "comb_b": comb_b,
            "w_ihT": w_ihT,
            "w_hhT": w_hhT,
            "b_ih": b_ih,
            "b_hh": b_hh,
            "fc_w8": _pack_fp8(_pad_cols(np.ascontiguousarray(fc_w_pad[v0:v0 + VS].T), VSP)),
            "fc_b": _pad_cols(fc_b_pad[v0:v0 + VS].reshape(1, VS), VSP, fill=-1e30).astype(bf),
        })
    return in_maps


def _run(inputs, trace=False, trace_cores=None):
    no_bias = all(
        not np.any(np.asarray(inputs[k]))
        for k in ("attn_b", "comb_b", "b_ih", "b_hh", "fc_b"))
    key = ("nc", no_bias)
    if key not in _CACHE:
        _CACHE[key] = _build(no_bias)
    nc = _CACHE[key]
    in_maps = _stage(inputs)
    for c in range(NC):
        v0 = c * VS
        n_real = max(0, min(V - v0, VS))
        in_maps[c]["s_corr"] = np.full((128, 1), float(VSP - n_real)
                                       if no_bias else 0.0, np.float32)
        if no_bias:
            # pad columns rely on the s_corr subtraction, not a -inf bias
            in_maps[c]["fc_b"] = np.zeros_like(in_maps[c]["fc_b"])
    res = bass_utils.run_bass_kernel_spmd(
        nc, in_maps, core_ids=list(range(NC)), trace=trace, trace_cores=trace_cores)
    logp = np.concatenate([res.results[c]["out_logp"] for c in range(NC)], axis=1)[:, :V]
    h_new = np.concatenate([res.results[c]["out_h"] for c in range(NC)], axis=0)[None]
    attnw = np.concatenate([res.results[c]["out_attn"] for c in range(NC)], axis=0)
    return (logp, h_new, attnw), res


def kernel(**inputs):
    out, _ = _run(inputs, trace=False)
    return out


# revision 33
# speedup vs baseline: 1.1386x; 1.0771x over previous
"""AttnDecoderRNN single-step decoder on 8 Trainium2 NeuronCores.

Sharding:
  - Front (embedding gather, Bahdanau attention, combine+relu, GRU cell):
    data-parallel over batch (32 rows/core).
  - h_new all-gathered (bf16) across the 8 cores.
  - Final fc + log_softmax: tensor-parallel over the vocab dimension
    (6283 columns/core); log-softmax denominators all-gathered and the
    normalization applied locally.

Weights are pre-packed on the host at staging time (transposed to
[in, out] layout and cast to bf16) — a one-time model-load transform.
Activations (hidden, encoder_outputs, tokens) are staged untouched.
"""
import sys

if "/opt/trn_rl_repo" not in sys.path:
    sys.path.insert(0, "/opt/trn_rl_repo")

import numpy as np
import ml_dtypes

import concourse.bass as bass
import concourse.tile as tile
from concourse import bacc, mybir
from concourse import bass_utils
from concourse.masks import make_identity

BF16 = mybir.dt.bfloat16
FP8 = mybir.dt.float8e4
F32 = mybir.dt.float32
I32 = mybir.dt.int32
AF = mybir.ActivationFunctionType

H, V, B, L = 512, 50257, 256, 50
NC = 8
BS = B // NC            # 32 batch rows per core
VS = (V + NC - 1) // NC  # 6283 vocab columns per core
VSP = 6304               # VS padded to a multiple of 32 (SBUF row alignment)
VPAD = VS * NC           # 50264
KT = H // 128            # 4 contraction tiles of 128
RG = [list(range(NC))]

# vocab column tiles (PSUM bank limit: 512 f32 per matmul)
N_TILES = []
_off = 0
while _off < VSP:
    n = min(512, VSP - _off)
    N_TILES.append((_off, n))
    _off += n

_CACHE = {}


def _build(no_bias):
    nc = bacc.Bacc("TRN2", target_bir_lowering=False, debug=False, num_devices=NC)

    # ---- I/O ----
    tok = nc.dram_tensor("tok", [BS, 1], I32, kind="ExternalInput")
    h0 = nc.dram_tensor("h0", [BS, H], F32, kind="ExternalInput")
    enc = nc.dram_tensor("enc", [L, BS, H], F32, kind="ExternalInput")
    embt = nc.dram_tensor("embt", [V, H], BF16, kind="ExternalInput")
    attn_wT = nc.dram_tensor("attn_wT", [2 * H, L], BF16, kind="ExternalInput")
    attn_b = nc.dram_tensor("attn_b", [1, L], BF16, kind="ExternalInput")
    comb_wT = nc.dram_tensor("comb_wT", [2 * H, H], BF16, kind="ExternalInput")
    comb_b = nc.dram_tensor("comb_b", [1, H], BF16, kind="ExternalInput")
    w_ihT = nc.dram_tensor("w_ihT", [H, 3 * H], BF16, kind="ExternalInput")
    w_hhT = nc.dram_tensor("w_hhT", [H, 3 * H], BF16, kind="ExternalInput")
    b_ih = nc.dram_tensor("b_ih", [1, 3 * H], BF16, kind="ExternalInput")
    b_hh = nc.dram_tensor("b_hh", [1, 3 * H], BF16, kind="ExternalInput")
    fc_w8 = nc.dram_tensor("fc_w8", [128, 2, 2, VSP], FP8, kind="ExternalInput")
    fc_b = nc.dram_tensor("fc_b", [1, VSP], BF16, kind="ExternalInput")
    s_corr = nc.dram_tensor("s_corr", [128, 1], F32, kind="ExternalInput")

    out_logp = nc.dram_tensor("out_logp", [B, VS], F32, kind="ExternalOutput")
    out_h = nc.dram_tensor("out_h", [BS, H], F32, kind="ExternalOutput")
    out_attn = nc.dram_tensor("out_attn", [BS, L], F32, kind="ExternalOutput")

    cc_h_in = nc.dram_tensor("cc_h_in", [BS, H], BF16)
    cc_h_out = nc.dram_tensor("cc_h_out", [B, H], BF16, addr_space="Shared")
    cc_s_in = [nc.dram_tensor(f"cc_s_in{bt}", [1, 128], F32) for bt in range(2)]
    cc_s_out = [nc.dram_tensor(f"cc_s_out{bt}", [NC, 128], F32, addr_space="Shared")
                for bt in range(2)]
    cc_d_in = nc.dram_tensor("cc_d_in", [1, 8], F32)
    cc_d_out = nc.dram_tensor("cc_d_out", [NC, 8], F32, addr_space="Shared")

    with tile.TileContext(nc) as tc:
        with (
            tc.tile_pool(name="singles", bufs=1) as sg,
            tc.tile_pool(name="work", bufs=2) as wk,
            tc.tile_pool(name="encp", bufs=3) as encp,
            tc.tile_pool(name="encbp", bufs=4) as encbp,
            tc.tile_pool(name="outp", bufs=3) as outp,
            tc.tile_pool(name="trp", bufs=2, space="PSUM") as trp,
            tc.tile_pool(name="gp", bufs=3, space="PSUM") as gp,
            tc.tile_pool(name="zp", bufs=3, space="PSUM") as zp,
        ):
            # ---- constants ----
            id_bf = sg.tile([128, 128], BF16, tag="id_bf")
            make_identity(nc, id_bf[:])
            id_f = sg.tile([128, 128], F32, tag="id_f")
            make_identity(nc, id_f[:])
            ones_bf = sg.tile([1, 128], BF16, tag="ones")
            nc.vector.memset(ones_bf[:], 1.0)

            # ---- embedding gather ----
            tok_sb = sg.tile([BS, 1], I32, tag="tok")
            nc.sync.dma_start(out=tok_sb[:], in_=tok.ap())
            emb_own = sg.tile([BS, H], BF16, tag="embrow")
            nc.gpsimd.indirect_dma_start(
                out=emb_own[:], out_offset=None, in_=embt.ap(),
                in_offset=bass.IndirectOffsetOnAxis(ap=tok_sb[:, 0:1], axis=0),
            )
            h0_sb = sg.tile([BS, H], F32, tag="h0")
            nc.sync.dma_start(out=h0_sb[:], in_=h0.ap())

            aw_sb = sg.tile([128, 2 * KT, L], BF16, tag="aw")
            nc.sync.dma_start(out=aw_sb[:],
                              in_=attn_wT.ap().rearrange("(k p) l -> p k l", p=128))
            ab_sb = sg.tile([1, L], BF16, tag="ab")
            nc.sync.dma_start(out=ab_sb[:], in_=attn_b.ap())

            # ---- encoder outputs: 8 wide DMAs,
            # through a 4-deep rolling pool (16 batch rows buffered ahead) ----
            enc_sb = []
            enc_bf = []
            for c in range(8):
                et = encp.tile([L, 4 * H], F32, tag="enc", name=f"enc{c}")
                for half in range(2):
                    nc.sync.dma_start(
                        out=et[:, half * 2 * H:(half + 1) * 2 * H],
                        in_=enc.ap()[:, c * 4 + half * 2:c * 4 + (half + 1) * 2, :]
                        .rearrange("l b h -> l (b h)"))
                eb = encbp.tile([L, 4 * H], BF16, tag="encbf", name=f"encbf{c}")
                if c % 2 == 0:
                    nc.vector.tensor_copy(out=eb[:], in_=et[:])
                else:
                    nc.scalar.copy(out=eb[:], in_=et[:])
                enc_sb.append(et)
                enc_bf.append(eb)

            # ---- feature-major transposes of embedded and h0 ----
            embT = []
            h0T = []
            for k in range(KT):
                pe = trp.tile([128, BS], BF16, tag="tr")
                nc.tensor.transpose(out=pe[:], in_=emb_own[:, 128 * k:128 * (k + 1)],
                                    identity=id_bf[:BS, :BS])
                t = sg.tile([128, BS], BF16, tag=f"embT{k}")
                nc.vector.tensor_copy(out=t[:], in_=pe[:])
                embT.append(t)

                pf = trp.tile([128, BS], F32, tag="tr")
                nc.tensor.transpose(out=pf[:], in_=h0_sb[:, 128 * k:128 * (k + 1)],
                                    identity=id_f[:BS, :BS])
                t2 = sg.tile([128, BS], BF16, tag=f"h0T{k}")
                nc.vector.tensor_copy(out=t2[:], in_=pf[:])
                h0T.append(t2)

            # ---- attention scores + softmax ----
            psc = gp.tile([BS, 512], F32, tag="gpsum")
            for k in range(KT):
                nc.tensor.matmul(out=psc[:, :L], lhsT=embT[k][:], rhs=aw_sb[:, k, :],
                                 start=(k == 0), stop=False)
            for k in range(KT):
                nc.tensor.matmul(out=psc[:, :L], lhsT=h0T[k][:], rhs=aw_sb[:, KT + k, :],
                                 start=False, stop=(no_bias and k == KT - 1))
            if not no_bias:
                nc.tensor.matmul(out=psc[:, :L], lhsT=ones_bf[0:1, :BS], rhs=ab_sb[:],
                                 start=False, stop=True)

            # scores are tiny (|s| < ~1), exp is safe without max subtraction
            e_sb = sg.tile([BS, L], F32, tag="esb")
            ssum = sg.tile([BS, 1], F32, tag="ssum")
            nc.scalar.activation(out=e_sb[:], in_=psc[:, :L], func=AF.Exp,
                                 accum_out=ssum[:])
            rinv = sg.tile([BS, 1], F32, tag="rinv")
            nc.vector.reciprocal(rinv[:], ssum[:])
            # attn_weights output is produced off the critical path; the
            # einsum uses unnormalized exp scores and rescales its output.
            attnw = sg.tile([BS, L], F32, tag="attnw")
            nc.vector.tensor_scalar_mul(attnw[:], e_sb[:], rinv[:, 0:1])
            nc.sync.dma_start(out=out_attn.ap(), in_=attnw[:])

            # ---- attn_applied via masked accumulating matmuls ----
            pwt = trp.tile([L, BS], F32, tag="tr")
            nc.tensor.transpose(out=pwt[:], in_=e_sb[:], identity=id_f[:BS, :BS])
            wt_bf = sg.tile([L, BS], BF16, tag="wtbf")
            nc.vector.tensor_copy(out=wt_bf[:], in_=pwt[:])
            wmask = sg.tile([L, BS, BS], BF16, tag="wmask")
            nc.vector.tensor_copy(
                out=wmask[:],
                in_=wt_bf[:].rearrange("l (o j) -> l o j", o=1).to_broadcast([L, BS, BS]),
            )
            nc.gpsimd.affine_select(
                out=wmask[:], in_=wmask[:], compare_op=mybir.AluOpType.is_equal,
                fill=0.0, base=0, pattern=[[-1, BS], [1, BS]], channel_multiplier=0,
            )
            # dummy collective: absorbs the expensive first-collective setup
            # while the einsum/comb/GRU run, instead of on the h path. Placed
            # after the last front gpsimd op — the gpsimd FIFO blocks behind
            # a collective's completion wait.
            dmy = sg.tile([1, 8], F32, tag="dmy")
            nc.vector.memset(dmy[:], 0.0)
            nc.sync.dma_start(out=cc_d_in.ap(), in_=dmy[:])
            nc.gpsimd.collective_compute(
                "AllGather", mybir.AluOpType.bypass, replica_groups=RG,
                ins=[cc_d_in.ap()], outs=[cc_d_out.ap()],
            )
            patt = [gp.tile([BS, 512], F32, tag="gpsum", name=f"patt{i}")
                    for i in range(2)]
            for b2 in range(BS // 2):
                for i in range(2):
                    b = 2 * b2 + i
                    eb = enc_bf[b // 4]
                    bi = b % 4
                    nc.tensor.matmul(out=patt[i][:], lhsT=wmask[:, b, :],
                                     rhs=eb[:, bi * H:(bi + 1) * H],
                                     start=(b2 == 0), stop=(b2 == BS // 2 - 1))
            # merge the two chains; chain0 goes via ACT to SBUF (one PSUM
            # operand max per DVE op), then normalize by the softmax sum
            aa0 = sg.tile([BS, H], F32, tag="aa0")
            nc.scalar.copy(out=aa0[:], in_=patt[0][:])
            aa_s = sg.tile([BS, H], F32, tag="aas")
            nc.vector.tensor_add(out=aa_s[:], in0=aa0[:], in1=patt[1][:])
            aa_bf = sg.tile([BS, H], BF16, tag="aabf")
            nc.vector.tensor_scalar_mul(aa_bf[:], aa_s[:], rinv[:, 0:1])

            aaT = []
            for k in range(KT):
                pe = trp.tile([128, BS], BF16, tag="tr")
                nc.tensor.transpose(out=pe[:], in_=aa_bf[:, 128 * k:128 * (k + 1)],
                                    identity=id_bf[:BS, :BS])
                t = sg.tile([128, BS], BF16, tag=f"aaT{k}")
                nc.vector.tensor_copy(out=t[:], in_=pe[:])
                aaT.append(t)

            # ---- combine + relu ----
            cw_sb = sg.tile([128, 2 * KT, H], BF16, tag="cw")
            nc.sync.dma_start(out=cw_sb[:],
                              in_=comb_wT.ap().rearrange("(k p) n -> p k n", p=128))
            cb_sb = sg.tile([1, H], BF16, tag="cb")
            nc.sync.dma_start(out=cb_sb[:], in_=comb_b.ap())
            px = gp.tile([BS, 512], F32, tag="gpsum")
            for k in range(KT):
                nc.tensor.matmul(out=px[:, :H], lhsT=embT[k][:], rhs=cw_sb[:, k, :],
                                 start=(k == 0), stop=False)
            for k in range(KT):
                nc.tensor.matmul(out=px[:, :H], lhsT=aaT[k][:], rhs=cw_sb[:, KT + k, :],
                                 start=False, stop=(no_bias and k == KT - 1))
            if not no_bias:
                nc.tensor.matmul(out=px[:, :H], lhsT=ones_bf[0:1, :BS], rhs=cb_sb[:],
                                 start=False, stop=True)
            x_bf = sg.tile([BS, H], BF16, tag="xbf")
            nc.scalar.activation(out=x_bf[:], in_=px[:, :H], func=AF.Relu)

            xT = []
            for k in range(KT):
                pe = trp.tile([128, BS], BF16, tag="tr")
                nc.tensor.transpose(out=pe[:], in_=x_bf[:, 128 * k:128 * (k + 1)],
                                    identity=id_bf[:BS, :BS])
                t = sg.tile([128, BS], BF16, tag=f"xT{k}")
                nc.vector.tensor_copy(out=t[:], in_=pe[:])
                xT.append(t)

            # ---- GRU cell ----
            wih_sb = sg.tile([128, KT, 3 * H], BF16, tag="wih")
            nc.sync.dma_start(out=wih_sb[:],
                              in_=w_ihT.ap().rearrange("(k p) n -> p k n", p=128))
            whh_sb = sg.tile([128, KT, 3 * H], BF16, tag="whh")
            nc.sync.dma_start(out=whh_sb[:],
                              in_=w_hhT.ap().rearrange("(k p) n -> p k n", p=128))
            bih_sb = sg.tile([1, 3 * H], BF16, tag="bih")
            nc.sync.dma_start(out=bih_sb[:], in_=b_ih.ap())
            bhh_sb = sg.tile([1, 3 * H], BF16, tag="bhh")
            nc.sync.dma_start(out=bhh_sb[:], in_=b_hh.ap())

            # r and z gates: gi + gh is just a longer matmul accumulation into
            # one PSUM tile; sigmoid reads the PSUM directly.
            r_sb = sg.tile([BS, H], F32, tag="r")
            z_gate = sg.tile([BS, H], F32, tag="zg")
            n_sb = sg.tile([BS, H], F32, tag="n")
            hnew = sg.tile([BS, H], F32, tag="hnew")
            for j, gate_out in ((0, r_sb), (1, z_gate)):
                pg = gp.tile([BS, 512], F32, tag="gpsum")
                for k in range(KT):
                    nc.tensor.matmul(out=pg[:, :H], lhsT=xT[k][:],
                                     rhs=wih_sb[:, k, H * j:H * (j + 1)],
                                     start=(k == 0), stop=False)
                for k in range(KT):
                    nc.tensor.matmul(out=pg[:, :H], lhsT=h0T[k][:],
                                     rhs=whh_sb[:, k, H * j:H * (j + 1)],
                                     start=False, stop=(no_bias and k == KT - 1))
                if not no_bias:
                    nc.tensor.matmul(out=pg[:, :H], lhsT=ones_bf[0:1, :BS],
                                     rhs=bih_sb[:, H * j:H * (j + 1)],
                                     start=False, stop=False)
                    nc.tensor.matmul(out=pg[:, :H], lhsT=ones_bf[0:1, :BS],
                                     rhs=bhh_sb[:, H * j:H * (j + 1)],
                                     start=False, stop=True)
                nc.scalar.activation(out=gate_out[:], in_=pg[:, :H], func=AF.Sigmoid)

            # n gate: i_n and h_n must stay separate (r multiplies h_n only)
            pgi = zp.tile([BS, 512], F32, tag="zpsum", name="pgi_n")
            for k in range(KT):
                nc.tensor.matmul(out=pgi[:, :H], lhsT=xT[k][:],
                                 rhs=wih_sb[:, k, 2 * H:3 * H],
                                 start=(k == 0), stop=(no_bias and k == KT - 1))
            if not no_bias:
                nc.tensor.matmul(out=pgi[:, :H], lhsT=ones_bf[0:1, :BS],
                                 rhs=bih_sb[:, 2 * H:3 * H], start=False, stop=True)
            pgh = gp.tile([BS, 512], F32, tag="gpsum")
            for k in range(KT):
                nc.tensor.matmul(out=pgh[:, :H], lhsT=h0T[k][:],
                                 rhs=whh_sb[:, k, 2 * H:3 * H],
                                 start=(k == 0), stop=(no_bias and k == KT - 1))
            if not no_bias:
                nc.tensor.matmul(out=pgh[:, :H], lhsT=ones_bf[0:1, :BS],
                                 rhs=bhh_sb[:, 2 * H:3 * H], start=False, stop=True)
            hnr = sg.tile([BS, H], F32, tag="hnr")
            nc.vector.tensor_mul(out=hnr[:], in0=r_sb[:], in1=pgh[:, :H])
            pre = sg.tile([BS, H], F32, tag="pre2")
            nc.vector.tensor_add(out=pre[:], in0=hnr[:], in1=pgi[:, :H])
            nc.scalar.activation(out=n_sb[:], in_=pre[:], func=AF.Tanh)

            d_sb = sg.tile([BS, H], F32, tag="d")
            nc.vector.tensor_tensor(out=d_sb[:], in0=h0_sb[:], in1=n_sb[:],
                                    op=mybir.AluOpType.subtract)
            e2_sb = sg.tile([BS, H], F32, tag="e2")
            nc.vector.tensor_mul(out=e2_sb[:], in0=z_gate[:], in1=d_sb[:])
            # final add writes bf16 directly so the AllGather can fire without
            # an extra cast on the critical path; the f32 h_new output is
            # reconstructed from it off-path
            h_bf = sg.tile([BS, H], BF16, tag="hbf")
            nc.vector.tensor_add(out=h_bf[:], in0=n_sb[:], in1=e2_sb[:])
            nc.sync.dma_start(out=cc_h_in.ap(), in_=h_bf[:])
            nc.scalar.activation(out=hnew[:], in_=h_bf[:], func=AF.Copy)
            nc.sync.dma_start(out=out_h.ap(), in_=hnew[:])
            nc.gpsimd.collective_compute(
                "AllGather", mybir.AluOpType.bypass, replica_groups=RG,
                ins=[cc_h_in.ap()], outs=[cc_h_out.ap()],
            )

            hTp = [[sg.tile([128, 2, 128], FP8, tag=f"hTp{p}{bt}", name=f"hTp{p}{bt}")
                    for bt in range(2)] for p in range(2)]
            for bt in range(2):
                hf = wk.tile([128, H], BF16, tag="hfull")
                nc.sync.dma_start(out=hf[:], in_=cc_h_out.ap()[128 * bt:128 * (bt + 1), :])
                for k in range(KT):
                    pe = trp.tile([128, 128], BF16, tag="tr")
                    nc.tensor.transpose(out=pe[:], in_=hf[:, 128 * k:128 * (k + 1)],
                                        identity=id_bf[:])
                    nc.vector.tensor_scalar_mul(hTp[k // 2][bt][:, k % 2, :], pe[:],
                                                1.0 / 16.0)

            # ---- fc matmul + exp/σ stats ----
            # fc_wT is made fully SBUF-resident via 4 big DMAs that carry no
            # dependency on the front, so they stream during front + AllGather.
            if not no_bias:
                fcb_sb = sg.tile([1, VSP], BF16, tag="fcb")
                nc.sync.dma_start(out=fcb_sb[:], in_=fc_b.ap())
            scorr_sb = sg.tile([128, 1], F32, tag="scorr")
            nc.sync.dma_start(out=scorr_sb[:], in_=s_corr.ap())
            wz8 = sg.tile([128, 2, 2, VSP], FP8, tag="wz8")
            for pair in range(2):
                for j in range(2):
                    nc.sync.dma_start(out=wz8[:, pair, j, :],
                                      in_=fc_w8.ap()[:, pair, j, :])
            z_sb = [sg.tile([128, VSP], BF16, tag=f"z{bt}", name=f"z{bt}") for bt in range(2)]
            stats = [sg.tile([128, len(N_TILES)], F32, tag=f"st{bt}", name=f"stats{bt}") for bt in range(2)]

            # bt-outer: batch-tile 0 finishes its matmuls, fires its stats
            # AllGather, and normalizes+stores while batch-tile 1's matmuls
            # are still running on the PE.
            for bt in range(2):
                for ntp in range(0, len(N_TILES), 2):
                    grp = [(nt,) + N_TILES[nt]
                           for nt in range(ntp, min(ntp + 2, len(N_TILES)))]
                    pzs = {nt: zp.tile([128, 512], F32, tag="zpsum",
                                       name=f"pz{bt}_{nt}")
                           for nt, _, _ in grp}
                    for pair in range(2):
                        for nt, ncur, n in grp:
                            nc.tensor.matmul(out=pzs[nt][:, :n],
                                             lhsT=hTp[pair][bt][:],
                                             rhs=wz8[:, pair, :, ncur:ncur + n],
                                             start=(pair == 0),
                                             stop=(no_bias and pair == 1),
                                             perf_mode=mybir.MatmulPerfMode.DoubleRow)
                    for nt, ncur, n in grp:
                        if not no_bias:
                            nc.tensor.matmul(out=pzs[nt][:, :n], lhsT=ones_bf[0:1, :],
                                             rhs=fcb_sb[:, ncur:ncur + n], start=False,
                                             stop=True)
                        nc.vector.tensor_copy(out=z_sb[bt][:, ncur:ncur + n],
                                              in_=pzs[nt][:, :n])
                        esc = wk.tile([128, 512], BF16, tag="esc")
                        nc.scalar.activation(out=esc[:, :n], in_=pzs[nt][:, :n],
                                             func=AF.Exp,
                                             accum_out=stats[bt][:, nt:nt + 1])

                # local softmax denominator for this batch tile -> all-gather
                s_own = sg.tile([128, 1], F32, tag=f"sown{bt}", name=f"sown{bt}")
                nc.vector.reduce_sum(s_own[:], stats[bt][:, 0:len(N_TILES)],
                                     axis=mybir.AxisListType.X)
                if no_bias:
                    # zero-weight pad columns contribute exp(0)=1 each; remove
                    nc.vector.tensor_tensor(out=s_own[:], in0=s_own[:],
                                            in1=scorr_sb[:],
                                            op=mybir.AluOpType.subtract)
                pt = trp.tile([1, 128], F32, tag="tr")
                nc.tensor.transpose(out=pt[:], in_=s_own[:], identity=id_f[:])
                srow = sg.tile([1, 128], F32, tag=f"srow{bt}", name=f"srow{bt}")
                nc.vector.tensor_copy(out=srow[:], in_=pt[:])
                nc.sync.dma_start(out=cc_s_in[bt].ap(), in_=srow[:])
                nc.gpsimd.collective_compute(
                    "AllGather", mybir.AluOpType.bypass, replica_groups=RG,
                    ins=[cc_s_in[bt].ap()], outs=[cc_s_out[bt].ap()],
                )
                s_all = sg.tile([128, NC], F32, tag=f"sall{bt}", name=f"sall{bt}")
                nc.sync.dma_start(out=s_all[:],
                                  in_=cc_s_out[bt].ap().rearrange("r b -> b r"))
                s_tot = sg.tile([128, 1], F32, tag=f"stot{bt}", name=f"stot{bt}")
                nc.vector.reduce_sum(s_tot[:], s_all[:], axis=mybir.AxisListType.X)
                ls = sg.tile([128, 1], F32, tag=f"lse{bt}", name=f"lse{bt}")
                nc.scalar.activation(out=ls[:], in_=s_tot[:], func=AF.Ln)

                # normalize + store this batch tile in wide chunks
                ocur = 0
                while ocur < VS:
                    n = min(1024, VS - ocur)
                    o_t = outp.tile([128, 1024], F32, tag="ost")
                    nc.vector.tensor_scalar_sub(o_t[:, :n], z_sb[bt][:, ocur:ocur + n],
                                                ls[:, 0:1])
                    nc.sync.dma_start(
                        out=out_logp.ap()[128 * bt:128 * (bt + 1), ocur:ocur + n],
                        in_=o_t[:, :n])
                    ocur += n

    nc.compile()
    return nc


def _pack_fp8(wT):
    # wT [512, VSP] f32 -> [128, 2, 2, VSP] fp8e4m3, x16 scaling
    # (matmul uses h/16 so the scales cancel exactly in the f32 PSUM)
    arr = (wT * 16.0).reshape(2, 2, 128, wT.shape[1])  # [pair, j, ki, v]
    return np.ascontiguousarray(arr.transpose(2, 0, 1, 3)).astype(
        ml_dtypes.float8_e4m3)


def _pad_cols(a, w, fill=0.0):
    out = np.full((a.shape[0], w), fill, np.float32)
    out[:, :a.shape[1]] = a
    return out


def _stage(inputs):
    """Build the 8 per-core in_maps from the full-size inputs."""
    bf = ml_dtypes.bfloat16
    tok = np.asarray(inputs["input_tokens"]).astype(np.int32).reshape(B, 1)
    hidden = np.ascontiguousarray(np.asarray(inputs["hidden"], np.float32))[0]  # [B,H]
    enc = np.ascontiguousarray(np.asarray(inputs["encoder_outputs"], np.float32))
    emb_bf = np.asarray(inputs["emb"], np.float32).astype(bf)
    attn_wT = np.ascontiguousarray(np.asarray(inputs["attn_w"], np.float32).T).astype(bf)
    attn_b = np.asarray(inputs["attn_b"], np.float32).reshape(1, L).astype(bf)
    comb_wT = np.ascontiguousarray(np.asarray(inputs["comb_w"], np.float32).T).astype(bf)
    comb_b = np.asarray(inputs["comb_b"], np.float32).reshape(1, H).astype(bf)
    w_ihT = np.ascontiguousarray(np.asarray(inputs["w_ih"], np.float32).T).astype(bf)
    w_hhT = np.ascontiguousarray(np.asarray(inputs["w_hh"], np.float32).T).astype(bf)
    b_ih = np.asarray(inputs["b_ih"], np.float32).reshape(1, 3 * H).astype(bf)
    b_hh = np.asarray(inputs["b_hh"], np.float32).reshape(1, 3 * H).astype(bf)

    fc_w = np.asarray(inputs["fc_w"], np.float32)
    fc_b = np.asarray(inputs["fc_b"], np.float32)
    fc_w_pad = np.zeros((VPAD, H), np.float32)
    fc_w_pad[:V] = fc_w
    fc_b_pad = np.full((VPAD,), -1e30, np.float32)
    fc_b_pad[:V] = fc_b

    in_maps = []
    for c in range(NC):
        b0 = c * BS
        v0 = c * VS
        in_maps.append({
            "tok": tok[b0:b0 + BS],
            "h0": np.ascontiguousarray(hidden[b0:b0 + BS]),
            "enc": np.ascontiguousarray(enc[:, b0:b0 + BS, :]),
            "embt": emb_bf,
            "attn_wT": attn_wT,
            "attn_b": attn_b,
            "comb_wT": comb_wT,
            "comb_b": comb_b,
            "w_ihT": w_ihT,
            "w_hhT": w_hhT,
            "b_ih": b_ih,
            "b_hh": b_hh,
            "fc_w8": _pack_fp8(_pad_cols(np.ascontiguousarray(fc_w_pad[v0:v0 + VS].T), VSP)),
            "fc_b": _pad_cols(fc_b_pad[v0:v0 + VS].reshape(1, VS), VSP, fill=-1e30).astype(bf),
        })
    return in_maps


def _run(inputs, trace=False, trace_cores=None):
    no_bias = all(
        not np.any(np.asarray(inputs[k]))
        for k in ("attn_b", "comb_b", "b_ih", "b_hh", "fc_b"))
    key = ("nc", no_bias)
    if key not in _CACHE:
        _CACHE[key] = _build(no_bias)
    nc = _CACHE[key]
    in_maps = _stage(inputs)
    for c in range(NC):
        v0 = c * VS
        n_real = max(0, min(V - v0, VS))
        in_maps[c]["s_corr"] = np.full((128, 1), float(VSP - n_real)
                                       if no_bias else 0.0, np.float32)
        if no_bias:
            # pad columns rely on the s_corr subtraction, not a -inf bias
            in_maps[c]["fc_b"] = np.zeros_like(in_maps[c]["fc_b"])
    res = bass_utils.run_bass_kernel_spmd(
        nc, in_maps, core_ids=list(range(NC)), trace=trace, trace_cores=trace_cores)
    logp = np.concatenate([res.results[c]["out_logp"] for c in range(NC)], axis=1)[:, :V]
    h_new = np.concatenate([res.results[c]["out_h"] for c in range(NC)], axis=0)[None]
    attnw = np.concatenate([res.results[c]["out_attn"] for c in range(NC)], axis=0)
    return (logp, h_new, attnw), res


def kernel(**inputs):
    out, _ = _run(inputs, trace=False)
    return out


# revision 34
# speedup vs baseline: 1.2519x; 1.0995x over previous
"""AttnDecoderRNN single-step decoder on 8 Trainium2 NeuronCores.

Sharding:
  - Front (embedding gather, Bahdanau attention, combine+relu, GRU cell):
    data-parallel over batch (32 rows/core).
  - h_new all-gathered (bf16) across the 8 cores.
  - Final fc + log_softmax: tensor-parallel over the vocab dimension
    (6283 columns/core); log-softmax denominators all-gathered and the
    normalization applied locally.

Weights are pre-packed on the host at staging time (transposed to
[in, out] layout and cast to bf16) — a one-time model-load transform.
Activations (hidden, encoder_outputs, tokens) are staged untouched.
"""
import sys

if "/opt/trn_rl_repo" not in sys.path:
    sys.path.insert(0, "/opt/trn_rl_repo")

import numpy as np
import ml_dtypes

import concourse.bass as bass
import concourse.tile as tile
from concourse import bacc, mybir
from concourse import bass_utils
from concourse.masks import make_identity

BF16 = mybir.dt.bfloat16
FP8 = mybir.dt.float8e4
F32 = mybir.dt.float32
I32 = mybir.dt.int32
AF = mybir.ActivationFunctionType

H, V, B, L = 512, 50257, 256, 50
NC = 8
BS = B // NC            # 32 batch rows per core
VS = (V + NC - 1) // NC  # 6283 vocab columns per core
VSP = 6304               # VS padded to a multiple of 32 (SBUF row alignment)
VPAD = VS * NC           # 50264
KT = H // 128            # 4 contraction tiles of 128
RG = [list(range(NC))]

# vocab column tiles (PSUM bank limit: 512 f32 per matmul)
N_TILES = []
_off = 0
while _off < VSP:
    n = min(512, VSP - _off)
    N_TILES.append((_off, n))
    _off += n

_CACHE = {}
_DMASK = np.broadcast_to(np.eye(32, dtype=np.float32).reshape(1, 32 * 32),
                         (L, 32 * 32)).astype(ml_dtypes.bfloat16)


def _build(no_bias):
    nc = bacc.Bacc("TRN2", target_bir_lowering=False, debug=False, num_devices=NC)

    # ---- I/O ----
    tok = nc.dram_tensor("tok", [BS, 1], I32, kind="ExternalInput")
    h0 = nc.dram_tensor("h0", [BS, H], F32, kind="ExternalInput")
    enc = nc.dram_tensor("enc", [L, BS, H], F32, kind="ExternalInput")
    embt = nc.dram_tensor("embt", [V, H], BF16, kind="ExternalInput")
    attn_wT = nc.dram_tensor("attn_wT", [2 * H, L], BF16, kind="ExternalInput")
    attn_b = nc.dram_tensor("attn_b", [1, L], BF16, kind="ExternalInput")
    comb_wT = nc.dram_tensor("comb_wT", [2 * H, H], BF16, kind="ExternalInput")
    comb_b = nc.dram_tensor("comb_b", [1, H], BF16, kind="ExternalInput")
    w_ihT = nc.dram_tensor("w_ihT", [H, 3 * H], BF16, kind="ExternalInput")
    w_hhT = nc.dram_tensor("w_hhT", [H, 3 * H], BF16, kind="ExternalInput")
    b_ih = nc.dram_tensor("b_ih", [1, 3 * H], BF16, kind="ExternalInput")
    b_hh = nc.dram_tensor("b_hh", [1, 3 * H], BF16, kind="ExternalInput")
    fc_w8 = nc.dram_tensor("fc_w8", [128, 2, 2, VSP], FP8, kind="ExternalInput")
    fc_b = nc.dram_tensor("fc_b", [1, VSP], BF16, kind="ExternalInput")
    s_corr = nc.dram_tensor("s_corr", [128, 1], F32, kind="ExternalInput")
    dmask = nc.dram_tensor("dmask", [L, BS * BS], BF16, kind="ExternalInput")

    out_logp = nc.dram_tensor("out_logp", [B, VS], F32, kind="ExternalOutput")
    out_h = nc.dram_tensor("out_h", [BS, H], F32, kind="ExternalOutput")
    out_attn = nc.dram_tensor("out_attn", [BS, L], F32, kind="ExternalOutput")

    cc_h_in = nc.dram_tensor("cc_h_in", [BS, H], BF16)
    cc_h_out = nc.dram_tensor("cc_h_out", [B, H], BF16, addr_space="Shared")
    cc_s_in = [nc.dram_tensor(f"cc_s_in{bt}", [1, 128], F32) for bt in range(2)]
    cc_s_out = [nc.dram_tensor(f"cc_s_out{bt}", [NC, 128], F32, addr_space="Shared")
                for bt in range(2)]
    cc_d_in = nc.dram_tensor("cc_d_in", [1, 8], F32)
    cc_d_out = nc.dram_tensor("cc_d_out", [NC, 8], F32, addr_space="Shared")

    with tile.TileContext(nc) as tc:
        with (
            tc.tile_pool(name="singles", bufs=1) as sg,
            tc.tile_pool(name="work", bufs=2) as wk,
            tc.tile_pool(name="encp", bufs=3) as encp,
            tc.tile_pool(name="encbp", bufs=4) as encbp,
            tc.tile_pool(name="outp", bufs=3) as outp,
            tc.tile_pool(name="trp", bufs=2, space="PSUM") as trp,
            tc.tile_pool(name="gp", bufs=3, space="PSUM") as gp,
            tc.tile_pool(name="zp", bufs=3, space="PSUM") as zp,
        ):
            # ---- constants ----
            id_bf = sg.tile([128, 128], BF16, tag="id_bf")
            make_identity(nc, id_bf[:])
            id_f = sg.tile([128, 128], F32, tag="id_f")
            make_identity(nc, id_f[:])
            ones_bf = sg.tile([1, 128], BF16, tag="ones")
            nc.vector.memset(ones_bf[:], 1.0)
            dm_sb = sg.tile([L, BS * BS], BF16, tag="dmask")
            nc.sync.dma_start(out=dm_sb[:], in_=dmask.ap())

            # ---- embedding gather ----
            tok_sb = sg.tile([BS, 1], I32, tag="tok")
            nc.sync.dma_start(out=tok_sb[:], in_=tok.ap())
            emb_own = sg.tile([BS, H], BF16, tag="embrow")
            nc.gpsimd.indirect_dma_start(
                out=emb_own[:], out_offset=None, in_=embt.ap(),
                in_offset=bass.IndirectOffsetOnAxis(ap=tok_sb[:, 0:1], axis=0),
            )
            h0_sb = sg.tile([BS, H], F32, tag="h0")
            nc.sync.dma_start(out=h0_sb[:], in_=h0.ap())

            # dummy collective: fires by ~8us (after the last front gpsimd op,
            # whose FIFO blocks behind a collective completion) so the
            # expensive first-collective setup finishes before h is ready.
            dmy = sg.tile([1, 8], F32, tag="dmy")
            nc.vector.memset(dmy[:], 0.0)
            nc.sync.dma_start(out=cc_d_in.ap(), in_=dmy[:])
            nc.gpsimd.collective_compute(
                "AllGather", mybir.AluOpType.bypass, replica_groups=RG,
                ins=[cc_d_in.ap()], outs=[cc_d_out.ap()],
            )

            aw_sb = sg.tile([128, 2 * KT, L], BF16, tag="aw")
            nc.sync.dma_start(out=aw_sb[:],
                              in_=attn_wT.ap().rearrange("(k p) l -> p k l", p=128))
            ab_sb = sg.tile([1, L], BF16, tag="ab")
            nc.sync.dma_start(out=ab_sb[:], in_=attn_b.ap())

            # ---- encoder outputs: 8 wide DMAs,
            # through a 4-deep rolling pool (16 batch rows buffered ahead) ----
            enc_sb = []
            enc_bf = []
            for c in range(8):
                et = encp.tile([L, 4 * H], F32, tag="enc", name=f"enc{c}")
                for half in range(2):
                    nc.sync.dma_start(
                        out=et[:, half * 2 * H:(half + 1) * 2 * H],
                        in_=enc.ap()[:, c * 4 + half * 2:c * 4 + (half + 1) * 2, :]
                        .rearrange("l b h -> l (b h)"))
                eb = encbp.tile([L, 4 * H], BF16, tag="encbf", name=f"encbf{c}")
                if c % 2 == 0:
                    nc.vector.tensor_copy(out=eb[:], in_=et[:])
                else:
                    nc.scalar.copy(out=eb[:], in_=et[:])
                enc_sb.append(et)
                enc_bf.append(eb)

            # ---- feature-major transposes of embedded and h0 ----
            embT = []
            h0T = []
            for k in range(KT):
                pe = trp.tile([128, BS], BF16, tag="tr")
                nc.tensor.transpose(out=pe[:], in_=emb_own[:, 128 * k:128 * (k + 1)],
                                    identity=id_bf[:BS, :BS])
                t = sg.tile([128, BS], BF16, tag=f"embT{k}")
                nc.vector.tensor_copy(out=t[:], in_=pe[:])
                embT.append(t)

                pf = trp.tile([128, BS], F32, tag="tr")
                nc.tensor.transpose(out=pf[:], in_=h0_sb[:, 128 * k:128 * (k + 1)],
                                    identity=id_f[:BS, :BS])
                t2 = sg.tile([128, BS], BF16, tag=f"h0T{k}")
                nc.vector.tensor_copy(out=t2[:], in_=pf[:])
                h0T.append(t2)

            # ---- attention scores + softmax ----
            psc = gp.tile([BS, 512], F32, tag="gpsum")
            for k in range(KT):
                nc.tensor.matmul(out=psc[:, :L], lhsT=embT[k][:], rhs=aw_sb[:, k, :],
                                 start=(k == 0), stop=False)
            for k in range(KT):
                nc.tensor.matmul(out=psc[:, :L], lhsT=h0T[k][:], rhs=aw_sb[:, KT + k, :],
                                 start=False, stop=(no_bias and k == KT - 1))
            if not no_bias:
                nc.tensor.matmul(out=psc[:, :L], lhsT=ones_bf[0:1, :BS], rhs=ab_sb[:],
                                 start=False, stop=True)

            # scores are tiny (|s| < ~1), exp is safe without max subtraction
            e_sb = sg.tile([BS, L], F32, tag="esb")
            ssum = sg.tile([BS, 1], F32, tag="ssum")
            nc.scalar.activation(out=e_sb[:], in_=psc[:, :L], func=AF.Exp,
                                 accum_out=ssum[:])
            rinv = sg.tile([BS, 1], F32, tag="rinv")
            nc.vector.reciprocal(rinv[:], ssum[:])
            # attn_weights output is produced off the critical path; the
            # einsum uses unnormalized exp scores and rescales its output.
            attnw = sg.tile([BS, L], F32, tag="attnw")
            nc.vector.tensor_scalar_mul(attnw[:], e_sb[:], rinv[:, 0:1])
            nc.sync.dma_start(out=out_attn.ap(), in_=attnw[:])

            # ---- attn_applied via masked accumulating matmuls ----
            pwt = trp.tile([L, BS], F32, tag="tr")
            nc.tensor.transpose(out=pwt[:], in_=e_sb[:], identity=id_f[:BS, :BS])
            wt_bf = sg.tile([L, BS], BF16, tag="wtbf")
            nc.vector.tensor_copy(out=wt_bf[:], in_=pwt[:])
            wmask = sg.tile([L, BS, BS], BF16, tag="wmask")
            nc.vector.tensor_tensor(
                out=wmask[:],
                in0=wt_bf[:].rearrange("l (o j) -> l o j", o=1).to_broadcast([L, BS, BS]),
                in1=dm_sb[:].rearrange("l (i j) -> l i j", i=BS),
                op=mybir.AluOpType.mult,
            )
            patt = [gp.tile([BS, 512], F32, tag="gpsum", name=f"patt{i}")
                    for i in range(2)]
            for b2 in range(BS // 2):
                for i in range(2):
                    b = 2 * b2 + i
                    eb = enc_bf[b // 4]
                    bi = b % 4
                    nc.tensor.matmul(out=patt[i][:], lhsT=wmask[:, b, :],
                                     rhs=eb[:, bi * H:(bi + 1) * H],
                                     start=(b2 == 0), stop=(b2 == BS // 2 - 1))
            # merge the two chains; chain0 goes via ACT to SBUF (one PSUM
            # operand max per DVE op), then normalize by the softmax sum
            aa0 = sg.tile([BS, H], F32, tag="aa0")
            nc.scalar.copy(out=aa0[:], in_=patt[0][:])
            aa_s = sg.tile([BS, H], F32, tag="aas")
            nc.vector.tensor_add(out=aa_s[:], in0=aa0[:], in1=patt[1][:])
            aa_bf = sg.tile([BS, H], BF16, tag="aabf")
            nc.vector.tensor_scalar_mul(aa_bf[:], aa_s[:], rinv[:, 0:1])

            aaT = []
            for k in range(KT):
                pe = trp.tile([128, BS], BF16, tag="tr")
                nc.tensor.transpose(out=pe[:], in_=aa_bf[:, 128 * k:128 * (k + 1)],
                                    identity=id_bf[:BS, :BS])
                t = sg.tile([128, BS], BF16, tag=f"aaT{k}")
                nc.vector.tensor_copy(out=t[:], in_=pe[:])
                aaT.append(t)

            # ---- combine + relu ----
            cw_sb = sg.tile([128, 2 * KT, H], BF16, tag="cw")
            nc.sync.dma_start(out=cw_sb[:],
                              in_=comb_wT.ap().rearrange("(k p) n -> p k n", p=128))
            cb_sb = sg.tile([1, H], BF16, tag="cb")
            nc.sync.dma_start(out=cb_sb[:], in_=comb_b.ap())
            px = gp.tile([BS, 512], F32, tag="gpsum")
            for k in range(KT):
                nc.tensor.matmul(out=px[:, :H], lhsT=embT[k][:], rhs=cw_sb[:, k, :],
                                 start=(k == 0), stop=False)
            for k in range(KT):
                nc.tensor.matmul(out=px[:, :H], lhsT=aaT[k][:], rhs=cw_sb[:, KT + k, :],
                                 start=False, stop=(no_bias and k == KT - 1))
            if not no_bias:
                nc.tensor.matmul(out=px[:, :H], lhsT=ones_bf[0:1, :BS], rhs=cb_sb[:],
                                 start=False, stop=True)
            x_bf = sg.tile([BS, H], BF16, tag="xbf")
            nc.scalar.activation(out=x_bf[:], in_=px[:, :H], func=AF.Relu)

            xT = []
            for k in range(KT):
                pe = trp.tile([128, BS], BF16, tag="tr")
                nc.tensor.transpose(out=pe[:], in_=x_bf[:, 128 * k:128 * (k + 1)],
                                    identity=id_bf[:BS, :BS])
                t = sg.tile([128, BS], BF16, tag=f"xT{k}")
                nc.vector.tensor_copy(out=t[:], in_=pe[:])
                xT.append(t)

            # ---- GRU cell ----
            wih_sb = sg.tile([128, KT, 3 * H], BF16, tag="wih")
            nc.sync.dma_start(out=wih_sb[:],
                              in_=w_ihT.ap().rearrange("(k p) n -> p k n", p=128))
            whh_sb = sg.tile([128, KT, 3 * H], BF16, tag="whh")
            nc.sync.dma_start(out=whh_sb[:],
                              in_=w_hhT.ap().rearrange("(k p) n -> p k n", p=128))
            bih_sb = sg.tile([1, 3 * H], BF16, tag="bih")
            nc.sync.dma_start(out=bih_sb[:], in_=b_ih.ap())
            bhh_sb = sg.tile([1, 3 * H], BF16, tag="bhh")
            nc.sync.dma_start(out=bhh_sb[:], in_=b_hh.ap())

            # r and z gates: gi + gh is just a longer matmul accumulation into
            # one PSUM tile; sigmoid reads the PSUM directly.
            r_sb = sg.tile([BS, H], F32, tag="r")
            z_gate = sg.tile([BS, H], F32, tag="zg")
            n_sb = sg.tile([BS, H], F32, tag="n")
            hnew = sg.tile([BS, H], F32, tag="hnew")
            for j, gate_out in ((0, r_sb), (1, z_gate)):
                pg = gp.tile([BS, 512], F32, tag="gpsum")
                for k in range(KT):
                    nc.tensor.matmul(out=pg[:, :H], lhsT=xT[k][:],
                                     rhs=wih_sb[:, k, H * j:H * (j + 1)],
                                     start=(k == 0), stop=False)
                for k in range(KT):
                    nc.tensor.matmul(out=pg[:, :H], lhsT=h0T[k][:],
                                     rhs=whh_sb[:, k, H * j:H * (j + 1)],
                                     start=False, stop=(no_bias and k == KT - 1))
                if not no_bias:
                    nc.tensor.matmul(out=pg[:, :H], lhsT=ones_bf[0:1, :BS],
                                     rhs=bih_sb[:, H * j:H * (j + 1)],
                                     start=False, stop=False)
                    nc.tensor.matmul(out=pg[:, :H], lhsT=ones_bf[0:1, :BS],
                                     rhs=bhh_sb[:, H * j:H * (j + 1)],
                                     start=False, stop=True)
                nc.scalar.activation(out=gate_out[:], in_=pg[:, :H], func=AF.Sigmoid)

            # n gate: i_n and h_n must stay separate (r multiplies h_n only)
            pgi = zp.tile([BS, 512], F32, tag="zpsum", name="pgi_n")
            for k in range(KT):
                nc.tensor.matmul(out=pgi[:, :H], lhsT=xT[k][:],
                                 rhs=wih_sb[:, k, 2 * H:3 * H],
                                 start=(k == 0), stop=(no_bias and k == KT - 1))
            if not no_bias:
                nc.tensor.matmul(out=pgi[:, :H], lhsT=ones_bf[0:1, :BS],
                                 rhs=bih_sb[:, 2 * H:3 * H], start=False, stop=True)
            pgh = gp.tile([BS, 512], F32, tag="gpsum")
            for k in range(KT):
                nc.tensor.matmul(out=pgh[:, :H], lhsT=h0T[k][:],
                                 rhs=whh_sb[:, k, 2 * H:3 * H],
                                 start=(k == 0), stop=(no_bias and k == KT - 1))
            if not no_bias:
                nc.tensor.matmul(out=pgh[:, :H], lhsT=ones_bf[0:1, :BS],
                                 rhs=bhh_sb[:, 2 * H:3 * H], start=False, stop=True)
            hnr = sg.tile([BS, H], F32, tag="hnr")
            nc.vector.tensor_mul(out=hnr[:], in0=r_sb[:], in1=pgh[:, :H])
            pre = sg.tile([BS, H], F32, tag="pre2")
            nc.vector.tensor_add(out=pre[:], in0=hnr[:], in1=pgi[:, :H])
            nc.scalar.activation(out=n_sb[:], in_=pre[:], func=AF.Tanh)

            d_sb = sg.tile([BS, H], F32, tag="d")
            nc.vector.tensor_tensor(out=d_sb[:], in0=h0_sb[:], in1=n_sb[:],
                                    op=mybir.AluOpType.subtract)
            e2_sb = sg.tile([BS, H], F32, tag="e2")
            nc.vector.tensor_mul(out=e2_sb[:], in0=z_gate[:], in1=d_sb[:])
            # final add writes bf16 directly so the AllGather can fire without
            # an extra cast on the critical path; the f32 h_new output is
            # reconstructed from it off-path
            h_bf = sg.tile([BS, H], BF16, tag="hbf")
            nc.vector.tensor_add(out=h_bf[:], in0=n_sb[:], in1=e2_sb[:])
            nc.sync.dma_start(out=cc_h_in.ap(), in_=h_bf[:])
            nc.scalar.activation(out=hnew[:], in_=h_bf[:], func=AF.Copy)
            nc.sync.dma_start(out=out_h.ap(), in_=hnew[:])
            nc.gpsimd.collective_compute(
                "AllGather", mybir.AluOpType.bypass, replica_groups=RG,
                ins=[cc_h_in.ap()], outs=[cc_h_out.ap()],
            )

            hTp = [[sg.tile([128, 2, 128], FP8, tag=f"hTp{p}{bt}", name=f"hTp{p}{bt}")
                    for bt in range(2)] for p in range(2)]
            for bt in range(2):
                hf = wk.tile([128, H], BF16, tag="hfull")
                nc.sync.dma_start(out=hf[:], in_=cc_h_out.ap()[128 * bt:128 * (bt + 1), :])
                for k in range(KT):
                    pe = trp.tile([128, 128], BF16, tag="tr")
                    nc.tensor.transpose(out=pe[:], in_=hf[:, 128 * k:128 * (k + 1)],
                                        identity=id_bf[:])
                    nc.vector.tensor_scalar_mul(hTp[k // 2][bt][:, k % 2, :], pe[:],
                                                1.0 / 16.0)

            # ---- fc matmul + exp/σ stats ----
            # fc_wT is made fully SBUF-resident via 4 big DMAs that carry no
            # dependency on the front, so they stream during front + AllGather.
            if not no_bias:
                fcb_sb = sg.tile([1, VSP], BF16, tag="fcb")
                nc.sync.dma_start(out=fcb_sb[:], in_=fc_b.ap())
            scorr_sb = sg.tile([128, 1], F32, tag="scorr")
            nc.sync.dma_start(out=scorr_sb[:], in_=s_corr.ap())
            wz8 = sg.tile([128, 2, 2, VSP], FP8, tag="wz8")
            for pair in range(2):
                for j in range(2):
                    nc.sync.dma_start(out=wz8[:, pair, j, :],
                                      in_=fc_w8.ap()[:, pair, j, :])
            z_sb = [sg.tile([128, VSP], BF16, tag=f"z{bt}", name=f"z{bt}") for bt in range(2)]
            stats = [sg.tile([128, len(N_TILES)], F32, tag=f"st{bt}", name=f"stats{bt}") for bt in range(2)]

            # bt-outer: batch-tile 0 finishes its matmuls, fires its stats
            # AllGather, and normalizes+stores while batch-tile 1's matmuls
            # are still running on the PE.
            for bt in range(2):
                for ntp in range(0, len(N_TILES), 2):
                    grp = [(nt,) + N_TILES[nt]
                           for nt in range(ntp, min(ntp + 2, len(N_TILES)))]
                    pzs = {nt: zp.tile([128, 512], F32, tag="zpsum",
                                       name=f"pz{bt}_{nt}")
                           for nt, _, _ in grp}
                    for pair in range(2):
                        for nt, ncur, n in grp:
                            nc.tensor.matmul(out=pzs[nt][:, :n],
                                             lhsT=hTp[pair][bt][:],
                                             rhs=wz8[:, pair, :, ncur:ncur + n],
                                             start=(pair == 0),
                                             stop=(no_bias and pair == 1),
                                             perf_mode=mybir.MatmulPerfMode.DoubleRow)
                    for nt, ncur, n in grp:
                        if not no_bias:
                            nc.tensor.matmul(out=pzs[nt][:, :n], lhsT=ones_bf[0:1, :],
                                             rhs=fcb_sb[:, ncur:ncur + n], start=False,
                                             stop=True)
                        nc.vector.tensor_copy(out=z_sb[bt][:, ncur:ncur + n],
                                              in_=pzs[nt][:, :n])
                        esc = wk.tile([128, 512], BF16, tag="esc")
                        nc.scalar.activation(out=esc[:, :n], in_=pzs[nt][:, :n],
                                             func=AF.Exp,
                                             accum_out=stats[bt][:, nt:nt + 1])

                # local softmax denominator for this batch tile -> all-gather
                s_own = sg.tile([128, 1], F32, tag=f"sown{bt}", name=f"sown{bt}")
                nc.vector.reduce_sum(s_own[:], stats[bt][:, 0:len(N_TILES)],
                                     axis=mybir.AxisListType.X)
                if no_bias:
                    # zero-weight pad columns contribute exp(0)=1 each; remove
                    nc.vector.tensor_tensor(out=s_own[:], in0=s_own[:],
                                            in1=scorr_sb[:],
                                            op=mybir.AluOpType.subtract)
                pt = trp.tile([1, 128], F32, tag="tr")
                nc.tensor.transpose(out=pt[:], in_=s_own[:], identity=id_f[:])
                srow = sg.tile([1, 128], F32, tag=f"srow{bt}", name=f"srow{bt}")
                nc.vector.tensor_copy(out=srow[:], in_=pt[:])
                nc.sync.dma_start(out=cc_s_in[bt].ap(), in_=srow[:])
                nc.gpsimd.collective_compute(
                    "AllGather", mybir.AluOpType.bypass, replica_groups=RG,
                    ins=[cc_s_in[bt].ap()], outs=[cc_s_out[bt].ap()],
                )
                s_all = sg.tile([128, NC], F32, tag=f"sall{bt}", name=f"sall{bt}")
                nc.sync.dma_start(out=s_all[:],
                                  in_=cc_s_out[bt].ap().rearrange("r b -> b r"))
                s_tot = sg.tile([128, 1], F32, tag=f"stot{bt}", name=f"stot{bt}")
                nc.vector.reduce_sum(s_tot[:], s_all[:], axis=mybir.AxisListType.X)
                ls = sg.tile([128, 1], F32, tag=f"lse{bt}", name=f"lse{bt}")
                nc.scalar.activation(out=ls[:], in_=s_tot[:], func=AF.Ln)

                # normalize + store this batch tile in wide chunks
                ocur = 0
                while ocur < VS:
                    n = min(1024, VS - ocur)
                    o_t = outp.tile([128, 1024], F32, tag="ost")
                    nc.vector.tensor_scalar_sub(o_t[:, :n], z_sb[bt][:, ocur:ocur + n],
                                                ls[:, 0:1])
                    nc.sync.dma_start(
                        out=out_logp.ap()[128 * bt:128 * (bt + 1), ocur:ocur + n],
                        in_=o_t[:, :n])
                    ocur += n

    nc.compile()
    return nc


def _pack_fp8(wT):
    # wT [512, VSP] f32 -> [128, 2, 2, VSP] fp8e4m3, x16 scaling
    # (matmul uses h/16 so the scales cancel exactly in the f32 PSUM)
    arr = (wT * 16.0).reshape(2, 2, 128, wT.shape[1])  # [pair, j, ki, v]
    return np.ascontiguousarray(arr.transpose(2, 0, 1, 3)).astype(
        ml_dtypes.float8_e4m3)


def _pad_cols(a, w, fill=0.0):
    out = np.full((a.shape[0], w), fill, np.float32)
    out[:, :a.shape[1]] = a
    return out


def _stage(inputs):
    """Build the 8 per-core in_maps from the full-size inputs."""
    bf = ml_dtypes.bfloat16
    tok = np.asarray(inputs["input_tokens"]).astype(np.int32).reshape(B, 1)
    hidden = np.ascontiguousarray(np.asarray(inputs["hidden"], np.float32))[0]  # [B,H]
    enc = np.ascontiguousarray(np.asarray(inputs["encoder_outputs"], np.float32))
    emb_bf = np.asarray(inputs["emb"], np.float32).astype(bf)
    attn_wT = np.ascontiguousarray(np.asarray(inputs["attn_w"], np.float32).T).astype(bf)
    attn_b = np.asarray(inputs["attn_b"], np.float32).reshape(1, L).astype(bf)
    comb_wT = np.ascontiguousarray(np.asarray(inputs["comb_w"], np.float32).T).astype(bf)
    comb_b = np.asarray(inputs["comb_b"], np.float32).reshape(1, H).astype(bf)
    w_ihT = np.ascontiguousarray(np.asarray(inputs["w_ih"], np.float32).T).astype(bf)
    w_hhT = np.ascontiguousarray(np.asarray(inputs["w_hh"], np.float32).T).astype(bf)
    b_ih = np.asarray(inputs["b_ih"], np.float32).reshape(1, 3 * H).astype(bf)
    b_hh = np.asarray(inputs["b_hh"], np.float32).reshape(1, 3 * H).astype(bf)

    fc_w = np.asarray(inputs["fc_w"], np.float32)
    fc_b = np.asarray(inputs["fc_b"], np.float32)
    fc_w_pad = np.zeros((VPAD, H), np.float32)
    fc_w_pad[:V] = fc_w
    fc_b_pad = np.full((VPAD,), -1e30, np.float32)
    fc_b_pad[:V] = fc_b

    in_maps = []
    for c in range(NC):
        b0 = c * BS
        v0 = c * VS
        in_maps.append({
            "tok": tok[b0:b0 + BS],
            "h0": np.ascontiguousarray(hidden[b0:b0 + BS]),
            "enc": np.ascontiguousarray(enc[:, b0:b0 + BS, :]),
            "embt": emb_bf,
            "attn_wT": attn_wT,
            "attn_b": attn_b,
            "comb_wT": comb_wT,
            "comb_b": comb_b,
            "w_ihT": w_ihT,
            "w_hhT": w_hhT,
            "b_ih": b_ih,
            "b_hh": b_hh,
            "fc_w8": _pack_fp8(_pad_cols(np.ascontiguousarray(fc_w_pad[v0:v0 + VS].T), VSP)),
            "fc_b": _pad_cols(fc_b_pad[v0:v0 + VS].reshape(1, VS), VSP, fill=-1e30).astype(bf),
        })
    return in_maps


def _run(inputs, trace=False, trace_cores=None):
    no_bias = all(
        not np.any(np.asarray(inputs[k]))
        for k in ("attn_b", "comb_b", "b_ih", "b_hh", "fc_b"))
    key = ("nc", no_bias)
    if key not in _CACHE:
        _CACHE[key] = _build(no_bias)
    nc = _CACHE[key]
    in_maps = _stage(inputs)
    for c in range(NC):
        v0 = c * VS
        n_real = max(0, min(V - v0, VS))
        in_maps[c]["s_corr"] = np.full((128, 1), float(VSP - n_real)
                                       if no_bias else 0.0, np.float32)
        in_maps[c]["dmask"] = _DMASK
        if no_bias:
            # pad columns rely on the s_corr subtraction, not a -inf bias
            in_maps[c]["fc_b"] = np.zeros_like(in_maps[c]["fc_b"])
    res = bass_utils.run_bass_kernel_spmd(
        nc, in_maps, core_ids=list(range(NC)), trace=trace, trace_cores=trace_cores)
    logp = np.concatenate([res.results[c]["out_logp"] for c in range(NC)], axis=1)[:, :V]
    h_new = np.concatenate([res.results[c]["out_h"] for c in range(NC)], axis=0)[None]
    attnw = np.concatenate([res.results[c]["out_attn"] for c in range(NC)], axis=0)
    return (logp, h_new, attnw), res


def kernel(**inputs):
    out, _ = _run(inputs, trace=False)
    return out
